# revision 11
# baseline (speedup 1.0000x reference)
"""GQA sparse-attention kernel for 8 Trainium2 NeuronCores.

Sharding: data-parallel over batch (2) x sequence-parallel over query rows
(4 row-groups per batch, rows j::4 interleaved so causal work is balanced and
the program is SPMD-identical across cores). No collectives: each core holds
512 query rows and computes all 16 heads for them, then layernorm + out-proj
for its rows locally.

Per-core device pipeline (all activations kept transposed, contraction dim on
partitions):
  qT = (WqT/8).T-chunks @ xqT   [1024,512]   (scale 1/8 folded into Wq host-side)
  kT = WkT-chunks @ xkT         [256,2048]
  vT = WvT-chunks @ xvT -> PE-transpose -> v_ext [s,260] (4 kv-heads x (64+ones))
  simT[s,n] = kT_h.T-slices @ qT_g   (fp32r, 4 s-blocks merged per PSUM tile)
  P = exp(simT) * mask01  (ACT exp -> bf16, DVE multiply; mask = adj&causal,
                           pre-masked on host, converted on device)
  pv[65,n] += v_ext_h.T @ P  (ones column gives the softmax denominator)
  att = num/denom; all-masked rows fall back to mean over all v rows (matches
  the reference's uniform-softmax-over-finfo.min behavior exactly)
  LN stats via ones-vector matmuls over the E-on-partitions layout, then
  y = xnorm.T-chunks @ WoT (+bo via a K=1 ones matmul)
"""

import os
import sys

import numpy as np

for _p in ("/opt/trn_rl_repo", "/root/.axon_site/_ro/trn_rl_repo"):
    if os.path.isdir(_p) and _p not in sys.path:
        sys.path.insert(0, _p)

B, N, E = 2, 2048, 1024
HQ, HK, D = 16, 4, 64
G = HQ // HK          # 4 query heads per kv head
KVE = HK * D          # 256
NL = N // 4           # 512 local query rows per core
SB = N // 128         # 16 s-blocks
EC = E // 128         # 8 embedding chunks
LN_EPS = 1e-5
NEG = 0.0  # host pre-masks adj; no additive mask needed

_PROG_CACHE = {}


def build_program():
    import concourse.bass as bass
    import concourse.mybir as mybir
    import concourse.tile as tile
    from concourse import bacc

    dt = mybir.dt
    f32, f32r, bf16, i32 = dt.float32, dt.float32r, dt.bfloat16, dt.int32
    AF = mybir.ActivationFunctionType
    OP = mybir.AluOpType
    AX = mybir.AxisListType

    nc = bacc.Bacc("TRN2", target_bir_lowering=False, debug=False)

    def din(name, shape, dtp=f32):
        return nc.dram_tensor(name, shape, dtp, kind="ExternalInput").ap()

    xqT = din("xqT", [E, NL], f32r)
    xkT = din("xkT", [E, N], f32r)
    xvT = din("xvT", [E, N], f32r)
    adjT = din("adjT", [N, NL], i32)
    WqT = din("WqT", [E, E], f32r)          # pre-scaled by 1/8 on host
    WkT = din("WkT", [E, KVE], f32r)
    WvT = din("WvT", [E, KVE], f32r)
    WoT = din("WoT", [E, E], f32r)
    bq2d = din("bq2d", [EC, 128])     # bq/8
    bk2d = din("bk2d", [2, 128])
    bv2d = din("bv2d", [2, 128])
    bo1 = din("bo1", [1, E], f32r)
    lng = din("lng", [EC, 128])
    lnb = din("lnb", [EC, 128])
    ones1 = din("ones1", [1, 128], f32r)
    y = nc.dram_tensor("y", [NL, E], f32, kind="ExternalOutput").ap()

    r = lambda ap: ap  # tiles feeding fp32r matmuls are float32r-typed

    with tile.TileContext(nc) as tc, nc.allow_low_precision(
            "float32r operands for PE fast-path matmuls are intentional"):
        with (
            tc.tile_pool(name="const", bufs=1) as pc,
            tc.tile_pool(name="persist", bufs=1) as pp,
            tc.tile_pool(name="psc", bufs=1, space="PSUM") as psc,
        ):
            ident = pc.tile([128, 128], f32, tag="ident")
            from concourse.masks import make_identity
            make_identity(nc, ident[:])
            ones_k1 = pc.tile([1, 128], f32r, tag="ones_k1")
            nc.sync.dma_start(ones_k1[:], ones1)
            ones_m1 = pc.tile([128, 1], f32r, tag="ones_m1")
            nc.sync.dma_start(ones_m1[:], ones1)
            eps_c = pc.tile([1, 1], f32, tag="eps_c")
            nc.gpsimd.memset(eps_c[:], LN_EPS)

            # persistent tiles
            wo_sb = [pp.tile([128, E], f32r, tag=f"wo{e}", name=f"wo{e}") for e in range(EC)]
            for e in range(EC):
                nc.sync.dma_start(wo_sb[e][:], WoT[e * 128:(e + 1) * 128, :])
            bo_sb = pp.tile([1, E], f32r, tag="bo", name="bo")
            nc.sync.dma_start(bo_sb[:], bo1)
            lng_sb = [pp.tile([128, 1], f32, tag=f"lng{e}", name=f"lng{e}") for e in range(EC)]
            lnb_sb = [pp.tile([128, 1], f32, tag=f"lnb{e}", name=f"lnb{e}") for e in range(EC)]
            for e in range(EC):
                nc.sync.dma_start(lng_sb[e][:], lng[e:e + 1, :])
                nc.sync.dma_start(lnb_sb[e][:], lnb[e:e + 1, :])

            kT_sb = [pp.tile([128, N], f32r, tag=f"kt{m}", name=f"kt{m}") for m in range(2)]
            v_ext = [pp.tile([128, 4 * 65], bf16, tag=f"vx{k}", name=f"vx{k}") for k in range(SB)]
            meanv = [pp.tile([128, 1], f32, tag=f"mv{m}", name=f"mv{m}") for m in range(2)]
            mvh = [pp.tile([64, 1], f32, tag=f"mvh{h}", name=f"mvh{h}") for h in range(HK)]
            # q head-pair tiles: half 0 holds a head with even kv-head, half 1 odd,
            # so sim matmul operand partition bases match the kv-head's base in kT_sb.
            qp_sb = [pp.tile([128, NL], f32r, tag=f"qp{m}", name=f"qp{m}") for m in range(EC)]
            _EVEN = [0, 1, 2, 3, 8, 9, 10, 11]    # heads with (g//4) % 2 == 0
            _ODD = [4, 5, 6, 7, 12, 13, 14, 15]
            def _qslot(g):
                if (g // G) % 2 == 0:
                    return _EVEN.index(g), 0
                return _ODD.index(g), 1
            attT = [pp.tile([128, NL], f32r, tag=f"at{e}", name=f"at{e}") for e in range(EC)]
            # 6 merged mask tiles: (c=0,m=0..1), (c=1,m=0..3); cols = 4 blocks x 256
            mask_sb = {}
            for c, m in [(0, 0), (0, 1), (1, 0), (1, 1), (1, 2), (1, 3)]:
                mask_sb[(c, m)] = pp.tile([128, 1024], bf16, tag=f"mk{c}{m}", name=f"mk{c}{m}")

            # ---------------- phase 1: q projection ----------------
            with (
                tc.tile_pool(name="wq", bufs=1) as pwq,
                tc.tile_pool(name="xq", bufs=1) as pxq,
                tc.tile_pool(name="bq", bufs=1) as pbq,
                tc.tile_pool(name="psq", bufs=2, space="PSUM") as psq,
            ):
                wq_sb = [pwq.tile([128, E], f32r, tag=f"wq{e}", name=f"wq{e}") for e in range(EC)]
                xq_sb = [pxq.tile([128, NL], f32r, tag=f"xq{e}", name=f"xq{e}") for e in range(EC)]
                bq_sb = [pbq.tile([128, 1], f32, tag=f"bq{m}", name=f"bq{m}") for m in range(EC)]
                for e in range(EC):
                    nc.sync.dma_start(wq_sb[e][:], WqT[e * 128:(e + 1) * 128, :])
                    nc.sync.dma_start(xq_sb[e][:], xqT[e * 128:(e + 1) * 128, :])
                    nc.sync.dma_start(bq_sb[e][:], bq2d[e:e + 1, :])
                for mt in range(EC):
                    ps = psq.tile([128, NL], f32, tag="psq")
                    for e in range(EC):
                        nc.tensor.matmul(
                            ps[:], r(wq_sb[e][:, mt * 128:(mt + 1) * 128]),
                            r(xq_sb[e][:]), start=(e == 0), stop=(e == EC - 1))
                    for t in range(2):
                        g = 2 * mt + t
                        ti, slot = _qslot(g)
                        nc.scalar.activation(
                            qp_sb[ti][slot * 64:(slot + 1) * 64, :],
                            ps[t * 64:(t + 1) * 64, :], AF.Identity,
                            bias=bq_sb[mt][t * 64:(t + 1) * 64, :], scale=1.0)

            # ---------------- phase 2: k/v projections ----------------
            with (
                tc.tile_pool(name="wkv", bufs=1) as pwkv,
                tc.tile_pool(name="xkv", bufs=3) as pxkv,
                tc.tile_pool(name="vt", bufs=2) as pvt,
                tc.tile_pool(name="vs", bufs=1) as pvs,
                tc.tile_pool(name="pskv", bufs=2, space="PSUM") as pskv,
                tc.tile_pool(name="pst", bufs=2, space="PSUM") as pst,
            ):
                wk_sb = [pwkv.tile([128, KVE], f32r, tag=f"wk{e}", name=f"wk{e}") for e in range(EC)]
                wv_sb = [pwkv.tile([128, KVE], f32r, tag=f"wv{e}", name=f"wv{e}") for e in range(EC)]
                bk_sb = [pwkv.tile([128, 1], f32, tag=f"bk{m}", name=f"bk{m}") for m in range(2)]
                bv_sb = [pwkv.tile([128, 1], f32, tag=f"bv{m}", name=f"bv{m}") for m in range(2)]
                for e in range(EC):
                    nc.sync.dma_start(wk_sb[e][:], WkT[e * 128:(e + 1) * 128, :])
                    nc.sync.dma_start(wv_sb[e][:], WvT[e * 128:(e + 1) * 128, :])
                for m in range(2):
                    nc.sync.dma_start(bk_sb[m][:], bk2d[m:m + 1, :])
                    nc.sync.dma_start(bv_sb[m][:], bv2d[m:m + 1, :])
                vsum = [pvs.tile([128, 4], f32, tag=f"vsum{m}", name=f"vsum{m}") for m in range(2)]
                for st in range(4):  # s-tiles of 512
                    sl = slice(st * 512, (st + 1) * 512)
                    xk_sb = [pxkv.tile([128, 512], f32r, tag=f"xk{e % 2}", name=f"xk{e}") for e in range(EC)]
                    xv_sb = [pxkv.tile([128, 512], f32r, tag=f"xv{e % 2}", name=f"xv{e}") for e in range(EC)]
                    for e in range(EC):
                        nc.sync.dma_start(xk_sb[e][:], xkT[e * 128:(e + 1) * 128, sl])
                        nc.sync.dma_start(xv_sb[e][:], xvT[e * 128:(e + 1) * 128, sl])
                    for mt in range(2):
                        psk = pskv.tile([128, 512], f32, tag="psk")
                        for e in range(EC):
                            nc.tensor.matmul(
                                psk[:], r(wk_sb[e][:, mt * 128:(mt + 1) * 128]),
                                r(xk_sb[e][:]), start=(e == 0), stop=(e == EC - 1))
                        nc.scalar.activation(kT_sb[mt][:, sl], psk[:], AF.Identity,
                                             bias=bk_sb[mt][:], scale=1.0)
                        psv = pskv.tile([128, 512], f32, tag="psv")
                        for e in range(EC):
                            nc.tensor.matmul(
                                psv[:], r(wv_sb[e][:, mt * 128:(mt + 1) * 128]),
                                r(xv_sb[e][:]), start=(e == 0), stop=(e == EC - 1))
                        vt = pvt.tile([128, 512], f32, tag="vt")
                        nc.scalar.activation(vt[:], psv[:], AF.Identity,
                                             bias=bv_sb[mt][:], scale=1.0)
                        nc.vector.reduce_sum(vsum[mt][:, st:st + 1], vt[:], axis=AX.X)
                        for ss in range(4):
                            k = st * 4 + ss
                            pt = pst.tile([128, 128], f32, tag="pt")
                            nc.tensor.transpose(pt[:], vt[:, ss * 128:(ss + 1) * 128],
                                                ident[:])
                            src = pt[:].rearrange("p (h x) -> p h x", h=2)
                            dst = v_ext[k][:].rearrange("p (h x) -> p h x", h=4)
                            nc.vector.tensor_copy(dst[:, 2 * mt:2 * mt + 2, 0:64], src)
                for k in range(SB):
                    one_col = v_ext[k][:].rearrange("p (h x) -> p h x", h=4)[:, :, 64:65]
                    nc.gpsimd.memset(one_col, 1.0)
                for m in range(2):
                    nc.vector.tensor_reduce(meanv[m][:], vsum[m][:], axis=AX.X,
                                            op=OP.add)
                    nc.vector.tensor_scalar_mul(meanv[m][:], meanv[m][:], 1.0 / N)
                for h in range(HK):
                    nc.vector.tensor_copy(
                        mvh[h][:], meanv[h // 2][(h % 2) * 64:(h % 2) * 64 + 64, :])

            # ---------------- phase 3: attention ----------------
            with (
                tc.tile_pool(name="adjs", bufs=2) as padj,
                tc.tile_pool(name="exps", bufs=3) as pex,
                tc.tile_pool(name="pvs_sb", bufs=2) as ppvs,
                tc.tile_pool(name="tiny", bufs=2) as ptiny,
                tc.tile_pool(name="pssim", bufs=2, space="PSUM") as pssim,
                tc.tile_pool(name="pspv", bufs=2, space="PSUM") as pspv,
                tc.tile_pool(name="psbc", bufs=1, space="PSUM") as psbc,
            ):
                # build merged masks: tile (c, m) cols b*256.. = adj block (4m+b), n-cols c*256..
                for (c, m), mk in mask_sb.items():
                    stg = padj.tile([128, 1024], i32, tag="adjstg")
                    for b in range(4):
                        k = 4 * m + b
                        nc.sync.dma_start(
                            stg[:, b * 256:(b + 1) * 256],
                            adjT[k * 128:(k + 1) * 128, c * 256:(c + 1) * 256])
                    nc.vector.tensor_scalar(mk[:], stg[:], 0, None, op0=OP.not_equal)

                for g in range(HQ):
                    h = g // G
                    ti, slot = _qslot(g)
                    qg = qp_sb[ti][slot * 64:(slot + 1) * 64, :]
                    kh = kT_sb[h // 2][(h % 2) * 64:(h % 2) * 64 + 64, :]
                    pv = pspv.tile([65, 512], f32, tag="pv")
                    for c in range(2):
                        nm = 2 if c == 0 else 4
                        nsb = 4 * nm
                        for m in range(nm):
                            simp = pssim.tile([128, 1024], f32, tag="sim")
                            for b in range(4):
                                k = 4 * m + b
                                nc.tensor.matmul(
                                    simp[:, b * 256:(b + 1) * 256],
                                    r(kh[:, k * 128:(k + 1) * 128]),
                                    r(qg[:, c * 256:(c + 1) * 256]),
                                    start=True, stop=True)
                            ex = pex.tile([128, 1024], bf16, tag="ex")
                            nc.scalar.activation(ex[:], simp[:], AF.Exp)
                            nc.vector.tensor_tensor(ex[:], ex[:], mask_sb[(c, m)][:],
                                                    op=OP.mult)
                            for b in range(4):
                                k = 4 * m + b
                                nc.tensor.matmul(
                                    pv[:, c * 256:(c + 1) * 256],
                                    v_ext[k][:, 65 * h:65 * h + 65],
                                    ex[:, b * 256:(b + 1) * 256],
                                    start=(k == 0), stop=(k == nsb - 1))
                    # epilogue over both chunks at once
                    pvs = ppvs.tile([65, 512], f32, tag="pvs")
                    nc.vector.tensor_copy(pvs[:], pv[:])
                    den0 = ptiny.tile([1, 512], f32r, tag="den0")
                    em = ptiny.tile([1, 512], f32r, tag="em")
                    nc.vector.tensor_scalar(em[:], pvs[64:65, :], 0.0, None,
                                            op0=OP.is_equal)
                    nc.vector.tensor_copy(den0[:], pvs[64:65, :])
                    nc.vector.tensor_tensor(den0[:], den0[:], em[:], op=OP.add)
                    nc.vector.reciprocal(den0[:], den0[:])
                    bc_r = psbc.tile([64, 512], f32, tag="bcr")
                    nc.tensor.matmul(bc_r[:], r(ones_k1[0:1, 0:64]), r(den0[:]),
                                     start=True, stop=True)
                    bc_e = psbc.tile([64, 512], f32, tag="bce")
                    nc.tensor.matmul(bc_e[:], r(ones_k1[0:1, 0:64]), r(em[:]),
                                     start=True, stop=True)
                    # all ops at base 0, then one shifted copy into attT half
                    tmpa = ppvs.tile([64, 512], f32, tag="tmpa")
                    nc.vector.tensor_tensor(tmpa[:], pvs[0:64, :], bc_r[:], op=OP.mult)
                    nc.vector.scalar_tensor_tensor(tmpa[:], bc_e[:], mvh[h][:],
                                                   tmpa[:], op0=OP.mult, op1=OP.add)
                    att = attT[g // 2][(g % 2) * 64:(g % 2) * 64 + 64, :]
                    nc.vector.tensor_copy(att, tmpa[:])

            # ---------------- phase 4: layernorm + out projection ----------------
            with (
                tc.tile_pool(name="lnt", bufs=2) as plnt,
                tc.tile_pool(name="ysb", bufs=2) as pysb,
                tc.tile_pool(name="psst", bufs=1, space="PSUM") as psst,
                tc.tile_pool(name="pslb", bufs=1, space="PSUM") as pslb,
                tc.tile_pool(name="psy", bufs=2, space="PSUM") as psy,
            ):
                st_sum = psst.tile([1, NL], f32, tag="ssum")
                st_sq = psst.tile([1, NL], f32, tag="ssq")
                for e in range(EC):
                    nc.tensor.matmul(st_sum[:], r(ones_m1[:]), r(attT[e][:]),
                                     start=(e == 0), stop=(e == EC - 1))
                    sq = plnt.tile([128, NL], f32r, tag="sq")
                    nc.scalar.activation(sq[:], attT[e][:], AF.Square)
                    nc.tensor.matmul(st_sq[:], r(ones_m1[:]), r(sq[:]),
                                     start=(e == 0), stop=(e == EC - 1))
                mu = plnt.tile([1, NL], f32r, tag="mu")
                nc.vector.tensor_scalar_mul(mu[:], st_sum[:], 1.0 / E)
                var = plnt.tile([1, NL], f32, tag="var")
                nc.vector.tensor_scalar_mul(var[:], st_sq[:], 1.0 / E)
                mu2 = plnt.tile([1, NL], f32, tag="mu2")
                nc.vector.tensor_tensor(mu2[:], mu[:], mu[:], op=OP.mult)
                nc.vector.tensor_tensor(var[:], var[:], mu2[:], op=OP.subtract)
                sd = plnt.tile([1, NL], f32r, tag="sd")
                nc.scalar.activation(sd[:], var[:], AF.Sqrt, bias=eps_c[:])
                nc.vector.reciprocal(sd[:], sd[:])
                mb = pslb.tile([128, NL], f32, tag="mb")
                nc.tensor.matmul(mb[:], r(ones_k1[:]), r(mu[:]), start=True, stop=True)
                ib = pslb.tile([128, NL], f32, tag="ib")
                nc.tensor.matmul(ib[:], r(ones_k1[:]), r(sd[:]), start=True, stop=True)
                for e in range(EC):
                    tmp = plnt.tile([128, NL], f32, tag="xn")
                    nc.vector.tensor_tensor(tmp[:], attT[e][:], mb[:], op=OP.subtract)
                    nc.vector.tensor_tensor(tmp[:], tmp[:], ib[:], op=OP.mult)
                    nc.vector.tensor_scalar(attT[e][:], tmp[:], lng_sb[e][:],
                                            lnb_sb[e][:], op0=OP.mult, op1=OP.add)
                for nt in range(4):
                    for oc in range(2):
                        py = psy.tile([128, 512], f32, tag="py")
                        for e in range(EC):
                            nc.tensor.matmul(
                                py[:], r(attT[e][:, nt * 128:(nt + 1) * 128]),
                                r(wo_sb[e][:, oc * 512:(oc + 1) * 512]),
                                start=(e == 0), stop=False)
                        nc.tensor.matmul(py[:], r(ones_k1[:]),
                                         r(bo_sb[0:1, oc * 512:(oc + 1) * 512]),
                                         start=False, stop=True)
                        ys = pysb.tile([128, 512], f32, tag="ys")
                        nc.vector.tensor_copy(ys[:], py[:])
                        nc.sync.dma_start(
                            y[nt * 128:(nt + 1) * 128, oc * 512:(oc + 1) * 512],
                            ys[:])
    nc.finalize()
    return nc


def shard_inputs(inputs):
    q = np.asarray(inputs["query"], np.float32)
    k = np.asarray(inputs["key"], np.float32)
    v = np.asarray(inputs["value"], np.float32)
    adj = np.asarray(inputs["adj"], np.int32)
    WqT8 = np.ascontiguousarray(np.asarray(inputs["Wq"], np.float32).T) / np.float32(8.0)
    WkT = np.ascontiguousarray(np.asarray(inputs["Wk"], np.float32).T)
    WvT = np.ascontiguousarray(np.asarray(inputs["Wv"], np.float32).T)
    WoT = np.ascontiguousarray(np.asarray(inputs["Wo"], np.float32).T)
    bq8 = (np.asarray(inputs["bq"], np.float32) / np.float32(8.0)).reshape(EC, 128)
    bk2 = np.asarray(inputs["bk"], np.float32).reshape(2, 128)
    bv2 = np.asarray(inputs["bv"], np.float32).reshape(2, 128)
    bo1 = np.asarray(inputs["bo"], np.float32).reshape(1, E)
    lng = np.asarray(inputs["ln_g"], np.float32).reshape(EC, 128)
    lnb = np.asarray(inputs["ln_b"], np.float32).reshape(EC, 128)

    shared = dict(WqT=WqT8, WkT=WkT, WvT=WvT, WoT=WoT, bq2d=bq8, bk2d=bk2,
                  bv2d=bv2, bo1=bo1, lng=lng, lnb=lnb,
                  ones1=np.ones((1, 128), np.float32))
    per_b = []
    s_idx = np.arange(N)
    for b in range(B):
        per_b.append((np.ascontiguousarray(k[b].T), np.ascontiguousarray(v[b].T)))
    in_maps = []
    for c in range(8):
        b, j = divmod(c, 4)
        rows = np.arange(j, N, 4)
        causal = s_idx[None, :] <= rows[:, None]          # [NL, N]
        adjc = np.where(causal, adj[b][rows], 0)
        m = dict(shared)
        m["xqT"] = np.ascontiguousarray(q[b][rows].T)
        m["xkT"], m["xvT"] = per_b[b]
        m["adjT"] = np.ascontiguousarray(adjc.T.astype(np.int32))
        in_maps.append(m)
    return in_maps


def unshard_outputs(results):
    out = np.empty((B, N, E), np.float32)
    for c in range(8):
        b, j = divmod(c, 4)
        out[b, j::4, :] = results[c]["y"]
    return out


def kernel(**inputs):
    from concourse.bass_utils import run_bass_kernel_spmd

    if "nc" not in _PROG_CACHE:
        _PROG_CACHE["nc"] = build_program()
    nc = _PROG_CACHE["nc"]
    in_maps = shard_inputs(inputs)
    res = run_bass_kernel_spmd(nc, in_maps, core_ids=list(range(8)))
    return unshard_outputs(res.results)


# revision 15
# speedup vs baseline: 1.0914x; 1.0914x over previous
"""GQA sparse-attention kernel for 8 Trainium2 NeuronCores.

Sharding: data-parallel over batch (2) x sequence-parallel over query rows
(4 row-groups per batch, rows j::4 interleaved so causal work is balanced and
the program is SPMD-identical across cores). No collectives: each core holds
512 query rows and computes all 16 heads for them, then layernorm + out-proj
for its rows locally.

Per-core device pipeline (all activations kept transposed, contraction dim on
partitions):
  qT = (WqT/8).T-chunks @ xqT   [1024,512]   (scale 1/8 folded into Wq host-side)
  kT = WkT-chunks @ xkT         [256,2048]
  vT = WvT-chunks @ xvT -> PE-transpose -> v_ext [s,260] (4 kv-heads x (64+ones))
  simT[s,n] = kT_h.T-slices @ qT_g   (fp32r, 4 s-blocks merged per PSUM tile)
  P = exp(simT) * mask01  (ACT exp -> bf16, DVE multiply; mask = adj&causal,
                           pre-masked on host, converted on device)
  pv[65,n] += v_ext_h.T @ P  (ones column gives the softmax denominator)
  att = num/denom; all-masked rows fall back to mean over all v rows (matches
  the reference's uniform-softmax-over-finfo.min behavior exactly)
  LN stats via ones-vector matmuls over the E-on-partitions layout, then
  y = xnorm.T-chunks @ WoT (+bo via a K=1 ones matmul)
"""

import os
import sys

import numpy as np

for _p in ("/opt/trn_rl_repo", "/root/.axon_site/_ro/trn_rl_repo"):
    if os.path.isdir(_p) and _p not in sys.path:
        sys.path.insert(0, _p)

B, N, E = 2, 2048, 1024
HQ, HK, D = 16, 4, 64
G = HQ // HK          # 4 query heads per kv head
KVE = HK * D          # 256
NL = N // 4           # 512 local query rows per core
SB = N // 128         # 16 s-blocks
EC = E // 128         # 8 embedding chunks
LN_EPS = 1e-5
NEG = 0.0  # host pre-masks adj; no additive mask needed

_PROG_CACHE = {}


def build_program():
    import concourse.bass as bass
    import concourse.mybir as mybir
    import concourse.tile as tile
    from concourse import bacc

    dt = mybir.dt
    f32, f32r, bf16, i32 = dt.float32, dt.float32r, dt.bfloat16, dt.int32
    AF = mybir.ActivationFunctionType
    OP = mybir.AluOpType
    AX = mybir.AxisListType

    nc = bacc.Bacc("TRN2", target_bir_lowering=False, debug=False)

    def din(name, shape, dtp=f32):
        return nc.dram_tensor(name, shape, dtp, kind="ExternalInput").ap()

    xqT = din("xqT", [E, NL], f32r)
    xkT = din("xkT", [E, N], f32r)
    xvT = din("xvT", [E, N], f32r)
    adjT = din("adjT", [N, NL], i32)
    WqT = din("WqT", [E, E], f32r)          # pre-scaled by 1/8 on host
    WkT = din("WkT", [E, KVE], f32r)
    WvT = din("WvT", [E, KVE], f32r)
    WoT = din("WoT", [E, E], f32r)
    bq2d = din("bq2d", [EC, 128])     # bq/8
    bk2d = din("bk2d", [2, 128])
    bv2d = din("bv2d", [2, 128])
    bo1 = din("bo1", [1, E], f32r)
    lng = din("lng", [EC, 128])
    lnb = din("lnb", [EC, 128])
    ones1 = din("ones1", [1, 128], f32r)
    y = nc.dram_tensor("y", [NL, E], f32, kind="ExternalOutput").ap()

    r = lambda ap: ap  # tiles feeding fp32r matmuls are float32r-typed

    with tile.TileContext(nc) as tc, nc.allow_low_precision(
            "float32r operands for PE fast-path matmuls are intentional"):
        with (
            tc.tile_pool(name="const", bufs=1) as pc,
            tc.tile_pool(name="persist", bufs=1) as pp,
            tc.tile_pool(name="psc", bufs=1, space="PSUM") as psc,
        ):
            ident = pc.tile([128, 128], f32, tag="ident")
            from concourse.masks import make_identity
            make_identity(nc, ident[:])
            ones_k1 = pc.tile([1, 128], f32r, tag="ones_k1")
            nc.sync.dma_start(ones_k1[:], ones1)
            ones_m1 = pc.tile([128, 1], f32r, tag="ones_m1")
            nc.sync.dma_start(ones_m1[:], ones1)
            eps_c = pc.tile([1, 1], f32, tag="eps_c")
            nc.gpsimd.memset(eps_c[:], LN_EPS)

            # persistent tiles
            wo_sb = [pp.tile([128, E], f32r, tag=f"wo{e}", name=f"wo{e}") for e in range(EC)]
            for e in range(EC):
                nc.sync.dma_start(wo_sb[e][:], WoT[e * 128:(e + 1) * 128, :])
            bo_sb = pp.tile([1, E], f32r, tag="bo", name="bo")
            nc.sync.dma_start(bo_sb[:], bo1)
            lng_sb = [pp.tile([128, 1], f32, tag=f"lng{e}", name=f"lng{e}") for e in range(EC)]
            lnb_sb = [pp.tile([128, 1], f32, tag=f"lnb{e}", name=f"lnb{e}") for e in range(EC)]
            for e in range(EC):
                nc.sync.dma_start(lng_sb[e][:], lng[e:e + 1, :])
                nc.sync.dma_start(lnb_sb[e][:], lnb[e:e + 1, :])

            kT_sb = [pp.tile([128, N], f32r, tag=f"kt{m}", name=f"kt{m}") for m in range(2)]
            v_ext = [pp.tile([128, 4 * 65], bf16, tag=f"vx{k}", name=f"vx{k}") for k in range(SB)]
            meanv = [pp.tile([128, 1], f32, tag=f"mv{m}", name=f"mv{m}") for m in range(2)]
            mv2 = [pp.tile([128, 1], f32, tag=f"mv2{h}", name=f"mv2{h}") for h in range(HK)]
            # q head-pair tiles: half 0 holds a head with even kv-head, half 1 odd,
            # so sim matmul operand partition bases match the kv-head's base in kT_sb.
            qp_sb = [pp.tile([128, NL], f32r, tag=f"qp{m}", name=f"qp{m}") for m in range(EC)]
            _EVEN = [0, 1, 2, 3, 8, 9, 10, 11]    # heads with (g//4) % 2 == 0
            _ODD = [4, 5, 6, 7, 12, 13, 14, 15]
            def _qslot(g):
                if (g // G) % 2 == 0:
                    return _EVEN.index(g), 0
                return _ODD.index(g), 1
            attT = [pp.tile([128, NL], f32r, tag=f"at{e}", name=f"at{e}") for e in range(EC)]
            # merged mask tiles: 4 pair tiles (s-blocks 2m,2m+1 over n 0:512) and
            # 2 quad tiles (s-blocks 8+4m..11+4m over n 256:512)
            mask_pair = [pp.tile([128, 1024], bf16, tag=f"mkp{m}", name=f"mkp{m}") for m in range(4)]
            mask_quad = [pp.tile([128, 1024], bf16, tag=f"mkq{m}", name=f"mkq{m}") for m in range(2)]

            # ---------------- phase 1: q projection ----------------
            with (
                tc.tile_pool(name="wq", bufs=1) as pwq,
                tc.tile_pool(name="xq", bufs=1) as pxq,
                tc.tile_pool(name="bq", bufs=1) as pbq,
                tc.tile_pool(name="psq", bufs=2, space="PSUM") as psq,
            ):
                wq_sb = [pwq.tile([128, E], f32r, tag=f"wq{e}", name=f"wq{e}") for e in range(EC)]
                xq_sb = [pxq.tile([128, NL], f32r, tag=f"xq{e}", name=f"xq{e}") for e in range(EC)]
                bq_sb = [pbq.tile([128, 1], f32, tag=f"bq{m}", name=f"bq{m}") for m in range(EC)]
                for e in range(EC):
                    nc.sync.dma_start(wq_sb[e][:], WqT[e * 128:(e + 1) * 128, :])
                    nc.sync.dma_start(xq_sb[e][:], xqT[e * 128:(e + 1) * 128, :])
                    nc.sync.dma_start(bq_sb[e][:], bq2d[e:e + 1, :])
                for mt in range(EC):
                    ps = psq.tile([128, NL], f32, tag="psq")
                    for e in range(EC):
                        nc.tensor.matmul(
                            ps[:], r(wq_sb[e][:, mt * 128:(mt + 1) * 128]),
                            r(xq_sb[e][:]), start=(e == 0), stop=(e == EC - 1))
                    for t in range(2):
                        g = 2 * mt + t
                        ti, slot = _qslot(g)
                        nc.scalar.activation(
                            qp_sb[ti][slot * 64:(slot + 1) * 64, :],
                            ps[t * 64:(t + 1) * 64, :], AF.Identity,
                            bias=bq_sb[mt][t * 64:(t + 1) * 64, :], scale=1.0)

            # ---------------- phase 2: k/v projections ----------------
            with (
                tc.tile_pool(name="wkv", bufs=1) as pwkv,
                tc.tile_pool(name="xkv", bufs=3) as pxkv,
                tc.tile_pool(name="vt", bufs=2) as pvt,
                tc.tile_pool(name="vs", bufs=1) as pvs,
                tc.tile_pool(name="pskv", bufs=2, space="PSUM") as pskv,
                tc.tile_pool(name="pst", bufs=2, space="PSUM") as pst,
            ):
                wk_sb = [pwkv.tile([128, KVE], f32r, tag=f"wk{e}", name=f"wk{e}") for e in range(EC)]
                wv_sb = [pwkv.tile([128, KVE], f32r, tag=f"wv{e}", name=f"wv{e}") for e in range(EC)]
                bk_sb = [pwkv.tile([128, 1], f32, tag=f"bk{m}", name=f"bk{m}") for m in range(2)]
                bv_sb = [pwkv.tile([128, 1], f32, tag=f"bv{m}", name=f"bv{m}") for m in range(2)]
                for e in range(EC):
                    nc.sync.dma_start(wk_sb[e][:], WkT[e * 128:(e + 1) * 128, :])
                    nc.sync.dma_start(wv_sb[e][:], WvT[e * 128:(e + 1) * 128, :])
                for m in range(2):
                    nc.sync.dma_start(bk_sb[m][:], bk2d[m:m + 1, :])
                    nc.sync.dma_start(bv_sb[m][:], bv2d[m:m + 1, :])
                vsum = [pvs.tile([128, 4], f32, tag=f"vsum{m}", name=f"vsum{m}") for m in range(2)]
                for st in range(4):  # s-tiles of 512
                    sl = slice(st * 512, (st + 1) * 512)
                    xk_sb = [pxkv.tile([128, 512], f32r, tag=f"xk{e % 2}", name=f"xk{e}") for e in range(EC)]
                    xv_sb = [pxkv.tile([128, 512], f32r, tag=f"xv{e % 2}", name=f"xv{e}") for e in range(EC)]
                    for e in range(EC):
                        nc.sync.dma_start(xk_sb[e][:], xkT[e * 128:(e + 1) * 128, sl])
                        nc.sync.dma_start(xv_sb[e][:], xvT[e * 128:(e + 1) * 128, sl])
                    for mt in range(2):
                        psk = pskv.tile([128, 512], f32, tag="psk")
                        for e in range(EC):
                            nc.tensor.matmul(
                                psk[:], r(wk_sb[e][:, mt * 128:(mt + 1) * 128]),
                                r(xk_sb[e][:]), start=(e == 0), stop=(e == EC - 1))
                        nc.scalar.activation(kT_sb[mt][:, sl], psk[:], AF.Identity,
                                             bias=bk_sb[mt][:], scale=1.0)
                        psv = pskv.tile([128, 512], f32, tag="psv")
                        for e in range(EC):
                            nc.tensor.matmul(
                                psv[:], r(wv_sb[e][:, mt * 128:(mt + 1) * 128]),
                                r(xv_sb[e][:]), start=(e == 0), stop=(e == EC - 1))
                        vt = pvt.tile([128, 512], f32, tag="vt")
                        nc.scalar.activation(vt[:], psv[:], AF.Identity,
                                             bias=bv_sb[mt][:], scale=1.0)
                        nc.vector.reduce_sum(vsum[mt][:, st:st + 1], vt[:], axis=AX.X)
                        for ss in range(4):
                            k = st * 4 + ss
                            pt = pst.tile([128, 128], f32, tag="pt")
                            nc.tensor.transpose(pt[:], vt[:, ss * 128:(ss + 1) * 128],
                                                ident[:])
                            src = pt[:].rearrange("p (h x) -> p h x", h=2)
                            dst = v_ext[k][:].rearrange("p (h x) -> p h x", h=4)
                            nc.vector.tensor_copy(dst[:, 2 * mt:2 * mt + 2, 0:64], src)
                for k in range(SB):
                    one_col = v_ext[k][:].rearrange("p (h x) -> p h x", h=4)[:, :, 64:65]
                    nc.gpsimd.memset(one_col, 1.0)
                for m in range(2):
                    nc.vector.tensor_reduce(meanv[m][:], vsum[m][:], axis=AX.X,
                                            op=OP.add)
                    nc.vector.tensor_scalar_mul(meanv[m][:], meanv[m][:], 1.0 / N)
                for h in range(HK):
                    src = meanv[h // 2][(h % 2) * 64:(h % 2) * 64 + 64, :]
                    nc.vector.tensor_copy(mv2[h][0:64, :], src)
                    nc.vector.tensor_copy(mv2[h][64:128, :], src)

            # ---------------- phase 3: attention ----------------
            with (
                tc.tile_pool(name="adjs", bufs=2) as padj,
                tc.tile_pool(name="exps", bufs=3) as pex,
                tc.tile_pool(name="pvs_sb", bufs=2) as ppvs,
                tc.tile_pool(name="tiny", bufs=2) as ptiny,
                tc.tile_pool(name="pssim", bufs=2, space="PSUM") as pssim,
                tc.tile_pool(name="pspv", bufs=2, space="PSUM") as pspv,
                tc.tile_pool(name="psbc", bufs=1, space="PSUM") as psbc,
            ):
                # build merged masks
                for m in range(4):
                    stg = padj.tile([128, 1024], i32, tag="adjstg", name="adjstg")
                    for b in range(2):
                        k = 2 * m + b
                        nc.sync.dma_start(
                            stg[:, b * 512:(b + 1) * 512],
                            adjT[k * 128:(k + 1) * 128, 0:512])
                    nc.vector.tensor_scalar(mask_pair[m][:], stg[:], 0, None,
                                            op0=OP.not_equal)
                for m in range(2):
                    stg = padj.tile([128, 1024], i32, tag="adjstg", name="adjstg")
                    for b in range(4):
                        k = 8 + 4 * m + b
                        nc.sync.dma_start(
                            stg[:, b * 256:(b + 1) * 256],
                            adjT[k * 128:(k + 1) * 128, 256:512])
                    nc.vector.tensor_scalar(mask_quad[m][:], stg[:], 0, None,
                                            op0=OP.not_equal)

                # row-emptiness is head-independent; filled from head 0's denom
                em_t = ppvs.tile([65, 512], f32r, tag="em_t", name="em_t")
                em_bc2 = ppvs.tile([128, 512], f32, tag="em_bc2", name="em_bc2")
                ones65 = ppvs.tile([65, 128], f32r, tag="ones65", name="ones65")
                nc.sync.dma_start(ones65[64:65, :], ones1)

                for g in range(HQ):
                    h = g // G
                    ti, slot = _qslot(g)
                    qg = qp_sb[ti][slot * 64:(slot + 1) * 64, :]
                    kh = kT_sb[h // 2][(h % 2) * 64:(h % 2) * 64 + 64, :]
                    pv = pspv.tile([65, 512], f32, tag="pv")
                    for m in range(4):      # s-block pairs, full n
                        simp = pssim.tile([128, 1024], f32, tag="sim")
                        for b in range(2):
                            k = 2 * m + b
                            nc.tensor.matmul(
                                simp[:, b * 512:(b + 1) * 512],
                                kh[:, k * 128:(k + 1) * 128], qg[:],
                                start=True, stop=True)
                        ex = pex.tile([128, 1024], bf16, tag="ex")
                        nc.scalar.activation(ex[:], simp[:], AF.Exp)
                        nc.vector.tensor_tensor(ex[:], ex[:], mask_pair[m][:],
                                                op=OP.mult)
                        for b in range(2):
                            k = 2 * m + b
                            nc.tensor.matmul(
                                pv[:], v_ext[k][:, 65 * h:65 * h + 65],
                                ex[:, b * 512:(b + 1) * 512],
                                start=(k == 0), stop=False,
                                skip_group_check=True)
                    for m in range(2):      # s-block quads, n 256:512 only
                        simp = pssim.tile([128, 1024], f32, tag="sim")
                        for b in range(4):
                            k = 8 + 4 * m + b
                            nc.tensor.matmul(
                                simp[:, b * 256:(b + 1) * 256],
                                kh[:, k * 128:(k + 1) * 128], qg[:, 256:512],
                                start=True, stop=True)
                        ex = pex.tile([128, 1024], bf16, tag="ex")
                        nc.scalar.activation(ex[:], simp[:], AF.Exp)
                        nc.vector.tensor_tensor(ex[:], ex[:], mask_quad[m][:],
                                                op=OP.mult)
                        for b in range(4):
                            k = 8 + 4 * m + b
                            nc.tensor.matmul(
                                pv[:, 256:512], v_ext[k][:, 65 * h:65 * h + 65],
                                ex[:, b * 256:(b + 1) * 256],
                                start=False, stop=(k == 15),
                                skip_group_check=True)

                    if g == 0:
                        nc.vector.tensor_scalar(em_t[64:65, :], pv[64:65, :], 0.0,
                                                None, op0=OP.is_equal)
                        bce = psbc.tile([128, 512], f32, tag="bce", name="bce")
                        nc.tensor.matmul(bce[:], ones65[64:65, :], em_t[64:65, :],
                                         start=True, stop=True)
                        nc.vector.tensor_copy(em_bc2[:], bce[:])
                    # den_safe = den + em  (both live at partition 64)
                    den = ptiny.tile([65, 512], f32r, tag="den")
                    nc.vector.tensor_tensor(den[64:65, :], pv[64:65, :],
                                            em_t[64:65, :], op=OP.add)
                    bc_r = psbc.tile([64, 512], f32, tag="bcr")
                    nc.tensor.matmul(bc_r[:], ones65[64:65, 0:64], den[64:65, :],
                                     start=True, stop=True)
                    rec = ppvs.tile([64, 512], f32, tag="rec")
                    nc.vector.reciprocal_approx_fast(rec[:], bc_r[:])
                    p0 = (g % 2) * 64
                    att = attT[g // 2][p0:p0 + 64, :]
                    nc.vector.tensor_tensor(att, pv[0:64, :], rec[:], op=OP.mult)
                    nc.vector.scalar_tensor_tensor(
                        att, em_bc2[p0:p0 + 64, :], mv2[h][p0:p0 + 64, :], att,
                        op0=OP.mult, op1=OP.add)

            # ---------------- phase 4: layernorm + out projection ----------------
            with (
                tc.tile_pool(name="lnt", bufs=2) as plnt,
                tc.tile_pool(name="ysb", bufs=2) as pysb,
                tc.tile_pool(name="psst", bufs=1, space="PSUM") as psst,
                tc.tile_pool(name="pslb", bufs=1, space="PSUM") as pslb,
                tc.tile_pool(name="psy", bufs=2, space="PSUM") as psy,
            ):
                st_sum = psst.tile([1, NL], f32, tag="ssum")
                st_sq = psst.tile([1, NL], f32, tag="ssq")
                for e in range(EC):
                    nc.tensor.matmul(st_sum[:], r(ones_m1[:]), r(attT[e][:]),
                                     start=(e == 0), stop=(e == EC - 1))
                    sq = plnt.tile([128, NL], f32r, tag="sq")
                    nc.scalar.activation(sq[:], attT[e][:], AF.Square)
                    nc.tensor.matmul(st_sq[:], r(ones_m1[:]), r(sq[:]),
                                     start=(e == 0), stop=(e == EC - 1))
                mu = plnt.tile([1, NL], f32r, tag="mu")
                nc.vector.tensor_scalar_mul(mu[:], st_sum[:], 1.0 / E)
                var = plnt.tile([1, NL], f32, tag="var")
                nc.vector.tensor_scalar_mul(var[:], st_sq[:], 1.0 / E)
                mu2 = plnt.tile([1, NL], f32, tag="mu2")
                nc.vector.tensor_tensor(mu2[:], mu[:], mu[:], op=OP.mult)
                nc.vector.tensor_tensor(var[:], var[:], mu2[:], op=OP.subtract)
                sd = plnt.tile([1, NL], f32r, tag="sd")
                nc.scalar.activation(sd[:], var[:], AF.Sqrt, bias=eps_c[:])
                nc.vector.reciprocal(sd[:], sd[:])
                mb = pslb.tile([128, NL], f32, tag="mb")
                nc.tensor.matmul(mb[:], r(ones_k1[:]), r(mu[:]), start=True, stop=True)
                ib = pslb.tile([128, NL], f32, tag="ib")
                nc.tensor.matmul(ib[:], r(ones_k1[:]), r(sd[:]), start=True, stop=True)
                for e in range(EC):
                    tmp = plnt.tile([128, NL], f32, tag="xn")
                    nc.vector.tensor_tensor(tmp[:], attT[e][:], mb[:], op=OP.subtract)
                    nc.vector.tensor_tensor(tmp[:], tmp[:], ib[:], op=OP.mult)
                    nc.vector.tensor_scalar(attT[e][:], tmp[:], lng_sb[e][:],
                                            lnb_sb[e][:], op0=OP.mult, op1=OP.add)
                for nt in range(4):
                    for oc in range(2):
                        py = psy.tile([128, 512], f32, tag="py")
                        for e in range(EC):
                            nc.tensor.matmul(
                                py[:], r(attT[e][:, nt * 128:(nt + 1) * 128]),
                                r(wo_sb[e][:, oc * 512:(oc + 1) * 512]),
                                start=(e == 0), stop=False)
                        nc.tensor.matmul(py[:], r(ones_k1[:]),
                                         r(bo_sb[0:1, oc * 512:(oc + 1) * 512]),
                                         start=False, stop=True)
                        ys = pysb.tile([128, 512], f32, tag="ys")
                        nc.vector.tensor_copy(ys[:], py[:])
                        nc.sync.dma_start(
                            y[nt * 128:(nt + 1) * 128, oc * 512:(oc + 1) * 512],
                            ys[:])
    nc.finalize()
    return nc


def shard_inputs(inputs):
    q = np.asarray(inputs["query"], np.float32)
    k = np.asarray(inputs["key"], np.float32)
    v = np.asarray(inputs["value"], np.float32)
    adj = np.asarray(inputs["adj"], np.int32)
    WqT8 = np.ascontiguousarray(np.asarray(inputs["Wq"], np.float32).T) / np.float32(8.0)
    WkT = np.ascontiguousarray(np.asarray(inputs["Wk"], np.float32).T)
    WvT = np.ascontiguousarray(np.asarray(inputs["Wv"], np.float32).T)
    WoT = np.ascontiguousarray(np.asarray(inputs["Wo"], np.float32).T)
    bq8 = (np.asarray(inputs["bq"], np.float32) / np.float32(8.0)).reshape(EC, 128)
    bk2 = np.asarray(inputs["bk"], np.float32).reshape(2, 128)
    bv2 = np.asarray(inputs["bv"], np.float32).reshape(2, 128)
    bo1 = np.asarray(inputs["bo"], np.float32).reshape(1, E)
    lng = np.asarray(inputs["ln_g"], np.float32).reshape(EC, 128)
    lnb = np.asarray(inputs["ln_b"], np.float32).reshape(EC, 128)

    shared = dict(WqT=WqT8, WkT=WkT, WvT=WvT, WoT=WoT, bq2d=bq8, bk2d=bk2,
                  bv2d=bv2, bo1=bo1, lng=lng, lnb=lnb,
                  ones1=np.ones((1, 128), np.float32))
    per_b = []
    s_idx = np.arange(N)
    for b in range(B):
        per_b.append((np.ascontiguousarray(k[b].T), np.ascontiguousarray(v[b].T)))
    in_maps = []
    for c in range(8):
        b, j = divmod(c, 4)
        rows = np.arange(j, N, 4)
        causal = s_idx[None, :] <= rows[:, None]          # [NL, N]
        adjc = np.where(causal, adj[b][rows], 0)
        m = dict(shared)
        m["xqT"] = np.ascontiguousarray(q[b][rows].T)
        m["xkT"], m["xvT"] = per_b[b]
        m["adjT"] = np.ascontiguousarray(adjc.T.astype(np.int32))
        in_maps.append(m)
    return in_maps


def unshard_outputs(results):
    out = np.empty((B, N, E), np.float32)
    for c in range(8):
        b, j = divmod(c, 4)
        out[b, j::4, :] = results[c]["y"]
    return out


def kernel(**inputs):
    from concourse.bass_utils import run_bass_kernel_spmd

    if "nc" not in _PROG_CACHE:
        _PROG_CACHE["nc"] = build_program()
    nc = _PROG_CACHE["nc"]
    in_maps = shard_inputs(inputs)
    res = run_bass_kernel_spmd(nc, in_maps, core_ids=list(range(8)))
    return unshard_outputs(res.results)


# revision 16
# speedup vs baseline: 1.1028x; 1.0105x over previous
"""GQA sparse-attention kernel for 8 Trainium2 NeuronCores.

Sharding: data-parallel over batch (2) x sequence-parallel over query rows
(4 row-groups per batch, rows j::4 interleaved so causal work is balanced and
the program is SPMD-identical across cores). No collectives: each core holds
512 query rows and computes all 16 heads for them, then layernorm + out-proj
for its rows locally.

Per-core device pipeline (all activations kept transposed, contraction dim on
partitions):
  qT = (WqT/8).T-chunks @ xqT   [1024,512]   (scale 1/8 folded into Wq host-side)
  kT = WkT-chunks @ xkT         [256,2048]
  vT = WvT-chunks @ xvT -> PE-transpose -> v_ext [s,260] (4 kv-heads x (64+ones))
  simT[s,n] = kT_h.T-slices @ qT_g   (fp32r, 4 s-blocks merged per PSUM tile)
  P = exp(simT) * mask01  (ACT exp -> bf16, DVE multiply; mask = adj&causal,
                           pre-masked on host, converted on device)
  pv[65,n] += v_ext_h.T @ P  (ones column gives the softmax denominator)
  att = num/denom; all-masked rows fall back to mean over all v rows (matches
  the reference's uniform-softmax-over-finfo.min behavior exactly)
  LN stats via ones-vector matmuls over the E-on-partitions layout, then
  y = xnorm.T-chunks @ WoT (+bo via a K=1 ones matmul)
"""

import os
import sys

import numpy as np

for _p in ("/opt/trn_rl_repo", "/root/.axon_site/_ro/trn_rl_repo"):
    if os.path.isdir(_p) and _p not in sys.path:
        sys.path.insert(0, _p)

B, N, E = 2, 2048, 1024
HQ, HK, D = 16, 4, 64
G = HQ // HK          # 4 query heads per kv head
KVE = HK * D          # 256
NL = N // 4           # 512 local query rows per core
SB = N // 128         # 16 s-blocks
EC = E // 128         # 8 embedding chunks
LN_EPS = 1e-5
NEG = 0.0  # host pre-masks adj; no additive mask needed

_PROG_CACHE = {}


def build_program():
    import concourse.bass as bass
    import concourse.mybir as mybir
    import concourse.tile as tile
    from concourse import bacc

    dt = mybir.dt
    f32, f32r, bf16, i32 = dt.float32, dt.float32r, dt.bfloat16, dt.int32
    AF = mybir.ActivationFunctionType
    OP = mybir.AluOpType
    AX = mybir.AxisListType

    nc = bacc.Bacc("TRN2", target_bir_lowering=False, debug=False)

    def din(name, shape, dtp=f32):
        return nc.dram_tensor(name, shape, dtp, kind="ExternalInput").ap()

    xqT = din("xqT", [E, NL], f32r)
    xkT = din("xkT", [E, N], f32r)
    xvT = din("xvT", [E, N], f32r)
    adjT = din("adjT", [N, NL], i32)
    WqT = din("WqT", [E, E], f32r)          # pre-scaled by 1/8 on host
    WkT = din("WkT", [E, KVE], f32r)
    WvT = din("WvT", [E, KVE], f32r)
    WoT = din("WoT", [E, E], f32r)
    bq2d = din("bq2d", [EC, 128])     # bq/8
    bk2d = din("bk2d", [2, 128])
    bv2d = din("bv2d", [2, 128])
    bo1 = din("bo1", [1, E], f32r)
    lng = din("lng", [EC, 128])
    lnb = din("lnb", [EC, 128])
    ones1 = din("ones1", [1, 128], f32r)
    y = nc.dram_tensor("y", [NL, E], f32, kind="ExternalOutput").ap()

    r = lambda ap: ap  # tiles feeding fp32r matmuls are float32r-typed

    with tile.TileContext(nc) as tc, nc.allow_low_precision(
            "float32r operands for PE fast-path matmuls are intentional"):
        with (
            tc.tile_pool(name="const", bufs=1) as pc,
            tc.tile_pool(name="persist", bufs=1) as pp,
            tc.tile_pool(name="psc", bufs=1, space="PSUM") as psc,
        ):
            ident = pc.tile([128, 128], f32, tag="ident")
            from concourse.masks import make_identity
            make_identity(nc, ident[:])
            ones_k1 = pc.tile([1, 128], f32r, tag="ones_k1")
            nc.sync.dma_start(ones_k1[:], ones1)
            ones_m1 = pc.tile([128, 1], f32r, tag="ones_m1")
            nc.sync.dma_start(ones_m1[:], ones1)
            eps_c = pc.tile([1, 1], f32, tag="eps_c")
            nc.gpsimd.memset(eps_c[:], LN_EPS)

            # persistent tiles
            wo_sb = [pp.tile([128, E], f32r, tag=f"wo{e}", name=f"wo{e}") for e in range(EC)]
            for e in range(EC):
                nc.sync.dma_start(wo_sb[e][:], WoT[e * 128:(e + 1) * 128, :])
            bo_sb = pp.tile([1, E], f32r, tag="bo", name="bo")
            nc.sync.dma_start(bo_sb[:], bo1)
            lng_sb = [pp.tile([128, 1], f32, tag=f"lng{e}", name=f"lng{e}") for e in range(EC)]
            lnb_sb = [pp.tile([128, 1], f32, tag=f"lnb{e}", name=f"lnb{e}") for e in range(EC)]
            for e in range(EC):
                nc.sync.dma_start(lng_sb[e][:], lng[e:e + 1, :])
                nc.sync.dma_start(lnb_sb[e][:], lnb[e:e + 1, :])

            kT_sb = [pp.tile([128, N], f32r, tag=f"kt{m}", name=f"kt{m}") for m in range(2)]
            v_ext = [pp.tile([128, 4 * 65], bf16, tag=f"vx{k}", name=f"vx{k}") for k in range(SB)]
            meanv = [pp.tile([128, 1], f32, tag=f"mv{m}", name=f"mv{m}") for m in range(2)]
            mv2 = [pp.tile([128, 1], f32, tag=f"mv2{h}", name=f"mv2{h}") for h in range(HK)]
            # q head-pair tiles: half 0 holds a head with even kv-head, half 1 odd,
            # so sim matmul operand partition bases match the kv-head's base in kT_sb.
            qp_sb = [pp.tile([128, NL], f32r, tag=f"qp{m}", name=f"qp{m}") for m in range(EC)]
            _EVEN = [0, 1, 2, 3, 8, 9, 10, 11]    # heads with (g//4) % 2 == 0
            _ODD = [4, 5, 6, 7, 12, 13, 14, 15]
            def _qslot(g):
                if (g // G) % 2 == 0:
                    return _EVEN.index(g), 0
                return _ODD.index(g), 1
            attT = [pp.tile([128, NL], f32r, tag=f"at{e}", name=f"at{e}") for e in range(EC)]
            # merged mask tiles: 4 pair tiles (s-blocks 2m,2m+1 over n 0:512) and
            # 2 quad tiles (s-blocks 8+4m..11+4m over n 256:512)
            mask_pair = [pp.tile([128, 1024], bf16, tag=f"mkp{m}", name=f"mkp{m}") for m in range(4)]
            mask_quad = [pp.tile([128, 1024], bf16, tag=f"mkq{m}", name=f"mkq{m}") for m in range(2)]

            # ---------------- phase 1: q projection ----------------
            with (
                tc.tile_pool(name="wq", bufs=1) as pwq,
                tc.tile_pool(name="xq", bufs=1) as pxq,
                tc.tile_pool(name="bq", bufs=1) as pbq,
                tc.tile_pool(name="psq", bufs=2, space="PSUM") as psq,
            ):
                wq_sb = [pwq.tile([128, E], f32r, tag=f"wq{e}", name=f"wq{e}") for e in range(EC)]
                xq_sb = [pxq.tile([128, NL], f32r, tag=f"xq{e}", name=f"xq{e}") for e in range(EC)]
                bq_sb = [pbq.tile([128, 1], f32, tag=f"bq{m}", name=f"bq{m}") for m in range(EC)]
                for e in range(EC):
                    nc.sync.dma_start(wq_sb[e][:], WqT[e * 128:(e + 1) * 128, :])
                    nc.sync.dma_start(xq_sb[e][:], xqT[e * 128:(e + 1) * 128, :])
                    nc.sync.dma_start(bq_sb[e][:], bq2d[e:e + 1, :])
                for mt2 in range(EC // 2):
                    psA = psq.tile([128, NL], f32, tag="psqA", name="psqA")
                    psB = psq.tile([128, NL], f32, tag="psqB", name="psqB")
                    for e in range(EC):
                        for mt, ps in ((2 * mt2, psA), (2 * mt2 + 1, psB)):
                            nc.tensor.matmul(
                                ps[:], r(wq_sb[e][:, mt * 128:(mt + 1) * 128]),
                                r(xq_sb[e][:]), start=(e == 0), stop=(e == EC - 1))
                    for mt, ps in ((2 * mt2, psA), (2 * mt2 + 1, psB)):
                     for t in range(2):
                        g = 2 * mt + t
                        ti, slot = _qslot(g)
                        nc.scalar.activation(
                            qp_sb[ti][slot * 64:(slot + 1) * 64, :],
                            ps[t * 64:(t + 1) * 64, :], AF.Identity,
                            bias=bq_sb[mt][t * 64:(t + 1) * 64, :], scale=1.0)

            # ---------------- phase 2: k/v projections ----------------
            with (
                tc.tile_pool(name="wkv", bufs=1) as pwkv,
                tc.tile_pool(name="xkv", bufs=3) as pxkv,
                tc.tile_pool(name="vt", bufs=2) as pvt,
                tc.tile_pool(name="vs", bufs=1) as pvs,
                tc.tile_pool(name="pskv", bufs=2, space="PSUM") as pskv,
                tc.tile_pool(name="pst", bufs=2, space="PSUM") as pst,
            ):
                wk_sb = [pwkv.tile([128, KVE], f32r, tag=f"wk{e}", name=f"wk{e}") for e in range(EC)]
                wv_sb = [pwkv.tile([128, KVE], f32r, tag=f"wv{e}", name=f"wv{e}") for e in range(EC)]
                bk_sb = [pwkv.tile([128, 1], f32, tag=f"bk{m}", name=f"bk{m}") for m in range(2)]
                bv_sb = [pwkv.tile([128, 1], f32, tag=f"bv{m}", name=f"bv{m}") for m in range(2)]
                for e in range(EC):
                    nc.sync.dma_start(wk_sb[e][:], WkT[e * 128:(e + 1) * 128, :])
                    nc.sync.dma_start(wv_sb[e][:], WvT[e * 128:(e + 1) * 128, :])
                for m in range(2):
                    nc.sync.dma_start(bk_sb[m][:], bk2d[m:m + 1, :])
                    nc.sync.dma_start(bv_sb[m][:], bv2d[m:m + 1, :])
                vsum = [pvs.tile([128, 4], f32, tag=f"vsum{m}", name=f"vsum{m}") for m in range(2)]
                for st in range(4):  # s-tiles of 512
                    sl = slice(st * 512, (st + 1) * 512)
                    xk_sb = [pxkv.tile([128, 512], f32r, tag=f"xk{e % 2}", name=f"xk{e}") for e in range(EC)]
                    xv_sb = [pxkv.tile([128, 512], f32r, tag=f"xv{e % 2}", name=f"xv{e}") for e in range(EC)]
                    for e in range(EC):
                        nc.sync.dma_start(xk_sb[e][:], xkT[e * 128:(e + 1) * 128, sl])
                        nc.sync.dma_start(xv_sb[e][:], xvT[e * 128:(e + 1) * 128, sl])
                    for mt in range(2):
                        psk = pskv.tile([128, 512], f32, tag="psk")
                        psv = pskv.tile([128, 512], f32, tag="psv")
                        for e in range(EC):
                            nc.tensor.matmul(
                                psk[:], r(wk_sb[e][:, mt * 128:(mt + 1) * 128]),
                                r(xk_sb[e][:]), start=(e == 0), stop=(e == EC - 1))
                            nc.tensor.matmul(
                                psv[:], r(wv_sb[e][:, mt * 128:(mt + 1) * 128]),
                                r(xv_sb[e][:]), start=(e == 0), stop=(e == EC - 1))
                        nc.scalar.activation(kT_sb[mt][:, sl], psk[:], AF.Identity,
                                             bias=bk_sb[mt][:], scale=1.0)
                        vt = pvt.tile([128, 512], f32, tag="vt")
                        nc.scalar.activation(vt[:], psv[:], AF.Identity,
                                             bias=bv_sb[mt][:], scale=1.0)
                        nc.vector.reduce_sum(vsum[mt][:, st:st + 1], vt[:], axis=AX.X)
                        for ss in range(4):
                            k = st * 4 + ss
                            pt = pst.tile([128, 128], f32, tag="pt")
                            nc.tensor.transpose(pt[:], vt[:, ss * 128:(ss + 1) * 128],
                                                ident[:])
                            src = pt[:].rearrange("p (h x) -> p h x", h=2)
                            dst = v_ext[k][:].rearrange("p (h x) -> p h x", h=4)
                            nc.vector.tensor_copy(dst[:, 2 * mt:2 * mt + 2, 0:64], src)
                for k in range(SB):
                    one_col = v_ext[k][:].rearrange("p (h x) -> p h x", h=4)[:, :, 64:65]
                    nc.gpsimd.memset(one_col, 1.0)
                for m in range(2):
                    nc.vector.tensor_reduce(meanv[m][:], vsum[m][:], axis=AX.X,
                                            op=OP.add)
                    nc.vector.tensor_scalar_mul(meanv[m][:], meanv[m][:], 1.0 / N)
                for h in range(HK):
                    src = meanv[h // 2][(h % 2) * 64:(h % 2) * 64 + 64, :]
                    nc.vector.tensor_copy(mv2[h][0:64, :], src)
                    nc.vector.tensor_copy(mv2[h][64:128, :], src)

            # ---------------- phase 3: attention ----------------
            with (
                tc.tile_pool(name="adjs", bufs=2) as padj,
                tc.tile_pool(name="exps", bufs=3) as pex,
                tc.tile_pool(name="pvs_sb", bufs=2) as ppvs,
                tc.tile_pool(name="tiny", bufs=2) as ptiny,
                tc.tile_pool(name="pssim", bufs=2, space="PSUM") as pssim,
                tc.tile_pool(name="pspv", bufs=2, space="PSUM") as pspv,
                tc.tile_pool(name="psbc", bufs=1, space="PSUM") as psbc,
            ):
                # build merged masks
                for m in range(4):
                    stg = padj.tile([128, 1024], i32, tag="adjstg", name="adjstg")
                    for b in range(2):
                        k = 2 * m + b
                        nc.sync.dma_start(
                            stg[:, b * 512:(b + 1) * 512],
                            adjT[k * 128:(k + 1) * 128, 0:512])
                    nc.vector.tensor_scalar(mask_pair[m][:], stg[:], 0, None,
                                            op0=OP.not_equal)
                for m in range(2):
                    stg = padj.tile([128, 1024], i32, tag="adjstg", name="adjstg")
                    for b in range(4):
                        k = 8 + 4 * m + b
                        nc.sync.dma_start(
                            stg[:, b * 256:(b + 1) * 256],
                            adjT[k * 128:(k + 1) * 128, 256:512])
                    nc.vector.tensor_scalar(mask_quad[m][:], stg[:], 0, None,
                                            op0=OP.not_equal)

                # row-emptiness is head-independent; filled from head 0's denom
                em_t = ppvs.tile([65, 512], f32r, tag="em_t", name="em_t")
                em_bc2 = ppvs.tile([128, 512], f32, tag="em_bc2", name="em_bc2")
                ones65 = ppvs.tile([65, 128], f32r, tag="ones65", name="ones65")
                nc.sync.dma_start(ones65[64:65, :], ones1)

                for g in range(HQ):
                    h = g // G
                    ti, slot = _qslot(g)
                    qg = qp_sb[ti][slot * 64:(slot + 1) * 64, :]
                    kh = kT_sb[h // 2][(h % 2) * 64:(h % 2) * 64 + 64, :]
                    pv = pspv.tile([65, 512], f32, tag="pv")
                    for m in range(4):      # s-block pairs, full n
                        simp = pssim.tile([128, 1024], f32, tag="sim")
                        for b in range(2):
                            k = 2 * m + b
                            nc.tensor.matmul(
                                simp[:, b * 512:(b + 1) * 512],
                                kh[:, k * 128:(k + 1) * 128], qg[:],
                                start=True, stop=True)
                        ex = pex.tile([128, 1024], bf16, tag="ex")
                        nc.scalar.activation(ex[:], simp[:], AF.Exp)
                        nc.vector.tensor_tensor(ex[:], ex[:], mask_pair[m][:],
                                                op=OP.mult)
                        for b in range(2):
                            k = 2 * m + b
                            nc.tensor.matmul(
                                pv[:], v_ext[k][:, 65 * h:65 * h + 65],
                                ex[:, b * 512:(b + 1) * 512],
                                start=(k == 0), stop=False,
                                skip_group_check=True)
                    for m in range(2):      # s-block quads, n 256:512 only
                        simp = pssim.tile([128, 1024], f32, tag="sim")
                        for b in (0, 2, 1, 3):
                            k = 8 + 4 * m + b
                            nc.tensor.matmul(
                                simp[:, b * 256:(b + 1) * 256],
                                kh[:, k * 128:(k + 1) * 128], qg[:, 256:512],
                                start=True, stop=True)
                        ex = pex.tile([128, 1024], bf16, tag="ex")
                        nc.scalar.activation(ex[:], simp[:], AF.Exp)
                        nc.vector.tensor_tensor(ex[:], ex[:], mask_quad[m][:],
                                                op=OP.mult)
                        for b in range(4):
                            k = 8 + 4 * m + b
                            nc.tensor.matmul(
                                pv[:, 256:512], v_ext[k][:, 65 * h:65 * h + 65],
                                ex[:, b * 256:(b + 1) * 256],
                                start=False, stop=(k == 15),
                                skip_group_check=True)

                    if g == 0:
                        nc.vector.tensor_scalar(em_t[64:65, :], pv[64:65, :], 0.0,
                                                None, op0=OP.is_equal)
                        bce = psbc.tile([128, 512], f32, tag="bce", name="bce")
                        nc.tensor.matmul(bce[:], ones65[64:65, :], em_t[64:65, :],
                                         start=True, stop=True)
                        nc.vector.tensor_copy(em_bc2[:], bce[:])
                    # den_safe = den + em  (both live at partition 64)
                    den = ptiny.tile([65, 512], f32r, tag="den")
                    nc.vector.tensor_tensor(den[64:65, :], pv[64:65, :],
                                            em_t[64:65, :], op=OP.add)
                    bc_r = psbc.tile([64, 512], f32, tag="bcr")
                    nc.tensor.matmul(bc_r[:], ones65[64:65, 0:64], den[64:65, :],
                                     start=True, stop=True)
                    rec = ppvs.tile([64, 512], f32, tag="rec")
                    nc.vector.reciprocal_approx_fast(rec[:], bc_r[:])
                    p0 = (g % 2) * 64
                    att = attT[g // 2][p0:p0 + 64, :]
                    nc.vector.tensor_tensor(att, pv[0:64, :], rec[:], op=OP.mult)
                    nc.vector.scalar_tensor_tensor(
                        att, em_bc2[p0:p0 + 64, :], mv2[h][p0:p0 + 64, :], att,
                        op0=OP.mult, op1=OP.add)

            # ---------------- phase 4: layernorm + out projection ----------------
            with (
                tc.tile_pool(name="lnt", bufs=2) as plnt,
                tc.tile_pool(name="ysb", bufs=2) as pysb,
                tc.tile_pool(name="psst", bufs=1, space="PSUM") as psst,
                tc.tile_pool(name="pslb", bufs=1, space="PSUM") as pslb,
                tc.tile_pool(name="psy", bufs=2, space="PSUM") as psy,
            ):
                st_sum = psst.tile([1, NL], f32, tag="ssum")
                st_sq = psst.tile([1, NL], f32, tag="ssq")
                for e in range(EC):
                    nc.tensor.matmul(st_sum[:], r(ones_m1[:]), r(attT[e][:]),
                                     start=(e == 0), stop=(e == EC - 1))
                    sq = plnt.tile([128, NL], f32r, tag="sq")
                    nc.scalar.activation(sq[:], attT[e][:], AF.Square)
                    nc.tensor.matmul(st_sq[:], r(ones_m1[:]), r(sq[:]),
                                     start=(e == 0), stop=(e == EC - 1))
                mu = plnt.tile([1, NL], f32r, tag="mu")
                nc.vector.tensor_scalar_mul(mu[:], st_sum[:], 1.0 / E)
                var = plnt.tile([1, NL], f32, tag="var")
                nc.vector.tensor_scalar_mul(var[:], st_sq[:], 1.0 / E)
                mu2 = plnt.tile([1, NL], f32, tag="mu2")
                nc.vector.tensor_tensor(mu2[:], mu[:], mu[:], op=OP.mult)
                nc.vector.tensor_tensor(var[:], var[:], mu2[:], op=OP.subtract)
                sd = plnt.tile([1, NL], f32r, tag="sd")
                nc.scalar.activation(sd[:], var[:], AF.Sqrt, bias=eps_c[:])
                nc.vector.reciprocal(sd[:], sd[:])
                mb = pslb.tile([128, NL], f32, tag="mb")
                nc.tensor.matmul(mb[:], r(ones_k1[:]), r(mu[:]), start=True, stop=True)
                ib = pslb.tile([128, NL], f32, tag="ib")
                nc.tensor.matmul(ib[:], r(ones_k1[:]), r(sd[:]), start=True, stop=True)
                for e in range(EC):
                    tmp = plnt.tile([128, NL], f32, tag="xn")
                    nc.vector.tensor_tensor(tmp[:], attT[e][:], mb[:], op=OP.subtract)
                    nc.vector.tensor_tensor(tmp[:], tmp[:], ib[:], op=OP.mult)
                    nc.vector.tensor_scalar(attT[e][:], tmp[:], lng_sb[e][:],
                                            lnb_sb[e][:], op0=OP.mult, op1=OP.add)
                for nt in range(4):
                    pyA = psy.tile([128, 512], f32, tag="pyA", name="pyA")
                    pyB = psy.tile([128, 512], f32, tag="pyB", name="pyB")
                    pys = (pyA, pyB)
                    for e in range(EC):
                        for oc in range(2):
                            nc.tensor.matmul(
                                pys[oc][:], r(attT[e][:, nt * 128:(nt + 1) * 128]),
                                r(wo_sb[e][:, oc * 512:(oc + 1) * 512]),
                                start=(e == 0), stop=False)
                    for oc in range(2):
                        nc.tensor.matmul(pys[oc][:], r(ones_k1[:]),
                                         r(bo_sb[0:1, oc * 512:(oc + 1) * 512]),
                                         start=False, stop=True)
                        ys = pysb.tile([128, 512], f32, tag="ys")
                        nc.vector.tensor_copy(ys[:], pys[oc][:])
                        nc.sync.dma_start(
                            y[nt * 128:(nt + 1) * 128, oc * 512:(oc + 1) * 512],
                            ys[:])
    nc.finalize()
    return nc


def shard_inputs(inputs):
    q = np.asarray(inputs["query"], np.float32)
    k = np.asarray(inputs["key"], np.float32)
    v = np.asarray(inputs["value"], np.float32)
    adj = np.asarray(inputs["adj"], np.int32)
    WqT8 = np.ascontiguousarray(np.asarray(inputs["Wq"], np.float32).T) / np.float32(8.0)
    WkT = np.ascontiguousarray(np.asarray(inputs["Wk"], np.float32).T)
    WvT = np.ascontiguousarray(np.asarray(inputs["Wv"], np.float32).T)
    WoT = np.ascontiguousarray(np.asarray(inputs["Wo"], np.float32).T)
    bq8 = (np.asarray(inputs["bq"], np.float32) / np.float32(8.0)).reshape(EC, 128)
    bk2 = np.asarray(inputs["bk"], np.float32).reshape(2, 128)
    bv2 = np.asarray(inputs["bv"], np.float32).reshape(2, 128)
    bo1 = np.asarray(inputs["bo"], np.float32).reshape(1, E)
    lng = np.asarray(inputs["ln_g"], np.float32).reshape(EC, 128)
    lnb = np.asarray(inputs["ln_b"], np.float32).reshape(EC, 128)

    shared = dict(WqT=WqT8, WkT=WkT, WvT=WvT, WoT=WoT, bq2d=bq8, bk2d=bk2,
                  bv2d=bv2, bo1=bo1, lng=lng, lnb=lnb,
                  ones1=np.ones((1, 128), np.float32))
    per_b = []
    s_idx = np.arange(N)
    for b in range(B):
        per_b.append((np.ascontiguousarray(k[b].T), np.ascontiguousarray(v[b].T)))
    in_maps = []
    for c in range(8):
        b, j = divmod(c, 4)
        rows = np.arange(j, N, 4)
        causal = s_idx[None, :] <= rows[:, None]          # [NL, N]
        adjc = np.where(causal, adj[b][rows], 0)
        m = dict(shared)
        m["xqT"] = np.ascontiguousarray(q[b][rows].T)
        m["xkT"], m["xvT"] = per_b[b]
        m["adjT"] = np.ascontiguousarray(adjc.T.astype(np.int32))
        in_maps.append(m)
    return in_maps


def unshard_outputs(results):
    out = np.empty((B, N, E), np.float32)
    for c in range(8):
        b, j = divmod(c, 4)
        out[b, j::4, :] = results[c]["y"]
    return out


def kernel(**inputs):
    from concourse.bass_utils import run_bass_kernel_spmd

    if "nc" not in _PROG_CACHE:
        _PROG_CACHE["nc"] = build_program()
    nc = _PROG_CACHE["nc"]
    in_maps = shard_inputs(inputs)
    res = run_bass_kernel_spmd(nc, in_maps, core_ids=list(range(8)))
    return unshard_outputs(res.results)


# revision 17
# speedup vs baseline: 1.3802x; 1.2515x over previous
"""GQA sparse-attention kernel for 8 Trainium2 NeuronCores.

Sharding: data-parallel over batch (2) x sequence-parallel over query rows
(4 row-groups per batch, rows j::4 interleaved so causal work is balanced and
the program is SPMD-identical across cores). No collectives: each core holds
512 query rows and computes all 16 heads for them, then layernorm + out-proj
for its rows locally.

Per-core device pipeline (all activations kept transposed, contraction dim on
partitions):
  qT = (WqT/8).T-chunks @ xqT   [1024,512]   (scale 1/8 folded into Wq host-side)
  kT = WkT-chunks @ xkT         [256,2048]
  vT = WvT-chunks @ xvT -> PE-transpose -> v_ext [s,260] (4 kv-heads x (64+ones))
  simT[s,n] = kT_h.T-slices @ qT_g   (fp32r, 4 s-blocks merged per PSUM tile)
  P = exp(simT) * mask01  (ACT exp -> bf16, DVE multiply; mask = adj&causal,
                           pre-masked on host, converted on device)
  pv[65,n] += v_ext_h.T @ P  (ones column gives the softmax denominator)
  att = num/denom; all-masked rows fall back to mean over all v rows (matches
  the reference's uniform-softmax-over-finfo.min behavior exactly)
  LN stats via ones-vector matmuls over the E-on-partitions layout, then
  y = xnorm.T-chunks @ WoT (+bo via a K=1 ones matmul)
"""

import os
import sys

import numpy as np

for _p in ("/opt/trn_rl_repo", "/root/.axon_site/_ro/trn_rl_repo"):
    if os.path.isdir(_p) and _p not in sys.path:
        sys.path.insert(0, _p)

B, N, E = 2, 2048, 1024
HQ, HK, D = 16, 4, 64
G = HQ // HK          # 4 query heads per kv head
KVE = HK * D          # 256
NL = N // 4           # 512 local query rows per core
SB = N // 128         # 16 s-blocks
EC = E // 128         # 8 embedding chunks
LN_EPS = 1e-5
NEG = 0.0  # host pre-masks adj; no additive mask needed

_PROG_CACHE = {}


def build_program():
    import concourse.bass as bass
    import concourse.mybir as mybir
    import concourse.tile as tile
    from concourse import bacc

    dt = mybir.dt
    f32, f32r, bf16, i32 = dt.float32, dt.float32r, dt.bfloat16, dt.int32
    f16 = dt.float16
    AF = mybir.ActivationFunctionType
    OP = mybir.AluOpType
    AX = mybir.AxisListType

    nc = bacc.Bacc("TRN2", target_bir_lowering=False, debug=False)

    def din(name, shape, dtp=f32):
        return nc.dram_tensor(name, shape, dtp, kind="ExternalInput").ap()

    xqT = din("xqT", [E, NL], f32r)
    xkT = din("xkT", [E, N], f32r)
    xvT = din("xvT", [E, N], f32r)
    adjT = din("adjT", [N, NL], i32)
    WqT = din("WqT", [E, E], f32r)          # pre-scaled by 1/8 on host
    WkT = din("WkT", [E, KVE], f32r)
    WvT = din("WvT", [E, KVE], f32r)
    WoT = din("WoT", [E, E], f32r)
    bq2d = din("bq2d", [EC, 128])     # bq/8
    bk2d = din("bk2d", [2, 128])
    bv2d = din("bv2d", [2, 128])
    bo1 = din("bo1", [1, E], f32r)
    lng = din("lng", [EC, 128])
    lnb = din("lnb", [EC, 128])
    ones1 = din("ones1", [1, 128], f32r)
    y = nc.dram_tensor("y", [NL, E], f32, kind="ExternalOutput").ap()

    r = lambda ap: ap  # tiles feeding fp32r matmuls are float32r-typed

    with tile.TileContext(nc) as tc, nc.allow_low_precision(
            "float32r operands for PE fast-path matmuls are intentional"):
        with (
            tc.tile_pool(name="const", bufs=1) as pc,
            tc.tile_pool(name="persist", bufs=1) as pp,
            tc.tile_pool(name="psc", bufs=1, space="PSUM") as psc,
        ):
            ident = pc.tile([128, 128], f32, tag="ident")
            from concourse.masks import make_identity
            make_identity(nc, ident[:])
            ones_k1 = pc.tile([1, 128], f32r, tag="ones_k1")
            nc.sync.dma_start(ones_k1[:], ones1)
            ones_m1 = pc.tile([128, 1], f32r, tag="ones_m1")
            nc.sync.dma_start(ones_m1[:], ones1)
            eps_c = pc.tile([1, 1], f32, tag="eps_c")
            nc.gpsimd.memset(eps_c[:], LN_EPS)

            # persistent tiles
            wo_sb = [pp.tile([128, E], f32r, tag=f"wo{e}", name=f"wo{e}") for e in range(EC)]
            for e in range(EC):
                nc.sync.dma_start(wo_sb[e][:], WoT[e * 128:(e + 1) * 128, :])
            bo_sb = pp.tile([1, E], f32r, tag="bo", name="bo")
            nc.sync.dma_start(bo_sb[:], bo1)
            lng_sb = [pp.tile([128, 1], f32, tag=f"lng{e}", name=f"lng{e}") for e in range(EC)]
            lnb_sb = [pp.tile([128, 1], f32, tag=f"lnb{e}", name=f"lnb{e}") for e in range(EC)]
            for e in range(EC):
                nc.sync.dma_start(lng_sb[e][:], lng[e:e + 1, :])
                nc.sync.dma_start(lnb_sb[e][:], lnb[e:e + 1, :])

            kT_sb = [pp.tile([128, N], f16, tag=f"kt{m}", name=f"kt{m}") for m in range(2)]
            v_ext = [pp.tile([128, 4 * 65], f16, tag=f"vx{k}", name=f"vx{k}") for k in range(SB)]
            meanv = [pp.tile([128, 1], f32, tag=f"mv{m}", name=f"mv{m}") for m in range(2)]
            mv2 = [pp.tile([128, 1], f32, tag=f"mv2{h}", name=f"mv2{h}") for h in range(HK)]
            # q head-pair tiles: half 0 holds a head with even kv-head, half 1 odd,
            # so sim matmul operand partition bases match the kv-head's base in kT_sb.
            qp_sb = [pp.tile([128, NL], f16, tag=f"qp{m}", name=f"qp{m}") for m in range(EC)]
            _EVEN = [0, 1, 2, 3, 8, 9, 10, 11]    # heads with (g//4) % 2 == 0
            _ODD = [4, 5, 6, 7, 12, 13, 14, 15]
            def _qslot(g):
                if (g // G) % 2 == 0:
                    return _EVEN.index(g), 0
                return _ODD.index(g), 1
            attT = [pp.tile([128, NL], f32r, tag=f"at{e}", name=f"at{e}") for e in range(EC)]
            # merged mask tiles: 4 pair tiles (s-blocks 2m,2m+1 over n 0:512) and
            # 2 quad tiles (s-blocks 8+4m..11+4m over n 256:512)
            mask_pair = [pp.tile([128, 1024], f16, tag=f"mkp{m}", name=f"mkp{m}") for m in range(4)]
            mask_quad = [pp.tile([128, 1024], f16, tag=f"mkq{m}", name=f"mkq{m}") for m in range(2)]

            # ---------------- phase 1: q projection ----------------
            with (
                tc.tile_pool(name="wq", bufs=1) as pwq,
                tc.tile_pool(name="xq", bufs=1) as pxq,
                tc.tile_pool(name="bq", bufs=1) as pbq,
                tc.tile_pool(name="psq", bufs=2, space="PSUM") as psq,
            ):
                wq_sb = [pwq.tile([128, E], f32r, tag=f"wq{e}", name=f"wq{e}") for e in range(EC)]
                xq_sb = [pxq.tile([128, NL], f32r, tag=f"xq{e}", name=f"xq{e}") for e in range(EC)]
                bq_sb = [pbq.tile([128, 1], f32, tag=f"bq{m}", name=f"bq{m}") for m in range(EC)]
                for e in range(EC):
                    nc.sync.dma_start(wq_sb[e][:], WqT[e * 128:(e + 1) * 128, :])
                    nc.sync.dma_start(xq_sb[e][:], xqT[e * 128:(e + 1) * 128, :])
                    nc.sync.dma_start(bq_sb[e][:], bq2d[e:e + 1, :])
                for mt2 in range(EC // 2):
                    psA = psq.tile([128, NL], f32, tag="psqA", name="psqA")
                    psB = psq.tile([128, NL], f32, tag="psqB", name="psqB")
                    for e in range(EC):
                        for mt, ps in ((2 * mt2, psA), (2 * mt2 + 1, psB)):
                            nc.tensor.matmul(
                                ps[:], r(wq_sb[e][:, mt * 128:(mt + 1) * 128]),
                                r(xq_sb[e][:]), start=(e == 0), stop=(e == EC - 1))
                    for mt, ps in ((2 * mt2, psA), (2 * mt2 + 1, psB)):
                     for t in range(2):
                        g = 2 * mt + t
                        ti, slot = _qslot(g)
                        nc.scalar.activation(
                            qp_sb[ti][slot * 64:(slot + 1) * 64, :],
                            ps[t * 64:(t + 1) * 64, :], AF.Identity,
                            bias=bq_sb[mt][t * 64:(t + 1) * 64, :], scale=1.0)

            # ---------------- phase 2: k/v projections ----------------
            with (
                tc.tile_pool(name="wkv", bufs=1) as pwkv,
                tc.tile_pool(name="xkv", bufs=3) as pxkv,
                tc.tile_pool(name="vt", bufs=2) as pvt,
                tc.tile_pool(name="vs", bufs=1) as pvs,
                tc.tile_pool(name="pskv", bufs=2, space="PSUM") as pskv,
                tc.tile_pool(name="pst", bufs=2, space="PSUM") as pst,
            ):
                wk_sb = [pwkv.tile([128, KVE], f32r, tag=f"wk{e}", name=f"wk{e}") for e in range(EC)]
                wv_sb = [pwkv.tile([128, KVE], f32r, tag=f"wv{e}", name=f"wv{e}") for e in range(EC)]
                bk_sb = [pwkv.tile([128, 1], f32, tag=f"bk{m}", name=f"bk{m}") for m in range(2)]
                bv_sb = [pwkv.tile([128, 1], f32, tag=f"bv{m}", name=f"bv{m}") for m in range(2)]
                for e in range(EC):
                    nc.sync.dma_start(wk_sb[e][:], WkT[e * 128:(e + 1) * 128, :])
                    nc.sync.dma_start(wv_sb[e][:], WvT[e * 128:(e + 1) * 128, :])
                for m in range(2):
                    nc.sync.dma_start(bk_sb[m][:], bk2d[m:m + 1, :])
                    nc.sync.dma_start(bv_sb[m][:], bv2d[m:m + 1, :])
                vsum = [pvs.tile([128, 4], f32, tag=f"vsum{m}", name=f"vsum{m}") for m in range(2)]
                for st in range(4):  # s-tiles of 512
                    sl = slice(st * 512, (st + 1) * 512)
                    xk_sb = [pxkv.tile([128, 512], f32r, tag=f"xk{e % 2}", name=f"xk{e}") for e in range(EC)]
                    xv_sb = [pxkv.tile([128, 512], f32r, tag=f"xv{e % 2}", name=f"xv{e}") for e in range(EC)]
                    for e in range(EC):
                        nc.sync.dma_start(xk_sb[e][:], xkT[e * 128:(e + 1) * 128, sl])
                        nc.sync.dma_start(xv_sb[e][:], xvT[e * 128:(e + 1) * 128, sl])
                    for mt in range(2):
                        psk = pskv.tile([128, 512], f32, tag="psk")
                        psv = pskv.tile([128, 512], f32, tag="psv")
                        for e in range(EC):
                            nc.tensor.matmul(
                                psk[:], r(wk_sb[e][:, mt * 128:(mt + 1) * 128]),
                                r(xk_sb[e][:]), start=(e == 0), stop=(e == EC - 1))
                            nc.tensor.matmul(
                                psv[:], r(wv_sb[e][:, mt * 128:(mt + 1) * 128]),
                                r(xv_sb[e][:]), start=(e == 0), stop=(e == EC - 1))
                        nc.scalar.activation(kT_sb[mt][:, sl], psk[:], AF.Identity,
                                             bias=bk_sb[mt][:], scale=1.0)
                        vt = pvt.tile([128, 512], f32, tag="vt")
                        nc.scalar.activation(vt[:], psv[:], AF.Identity,
                                             bias=bv_sb[mt][:], scale=1.0)
                        nc.vector.reduce_sum(vsum[mt][:, st:st + 1], vt[:], axis=AX.X)
                        for ss in range(4):
                            k = st * 4 + ss
                            pt = pst.tile([128, 128], f32, tag="pt")
                            nc.tensor.transpose(pt[:], vt[:, ss * 128:(ss + 1) * 128],
                                                ident[:])
                            src = pt[:].rearrange("p (h x) -> p h x", h=2)
                            dst = v_ext[k][:].rearrange("p (h x) -> p h x", h=4)
                            nc.vector.tensor_copy(dst[:, 2 * mt:2 * mt + 2, 0:64], src)
                for k in range(SB):
                    one_col = v_ext[k][:].rearrange("p (h x) -> p h x", h=4)[:, :, 64:65]
                    nc.gpsimd.memset(one_col, 1.0)
                for m in range(2):
                    nc.vector.tensor_reduce(meanv[m][:], vsum[m][:], axis=AX.X,
                                            op=OP.add)
                    nc.vector.tensor_scalar_mul(meanv[m][:], meanv[m][:], 1.0 / N)
                for h in range(HK):
                    src = meanv[h // 2][(h % 2) * 64:(h % 2) * 64 + 64, :]
                    nc.vector.tensor_copy(mv2[h][0:64, :], src)
                    nc.vector.tensor_copy(mv2[h][64:128, :], src)

            # ---------------- phase 3: attention ----------------
            with (
                tc.tile_pool(name="adjs", bufs=2) as padj,
                tc.tile_pool(name="exps", bufs=3) as pex,
                tc.tile_pool(name="pvs_sb", bufs=2) as ppvs,
                tc.tile_pool(name="tiny", bufs=2) as ptiny,
                tc.tile_pool(name="pssim", bufs=2, space="PSUM") as pssim,
                tc.tile_pool(name="pspv", bufs=2, space="PSUM") as pspv,
                tc.tile_pool(name="psbc", bufs=1, space="PSUM") as psbc,
            ):
                # build merged masks
                for m in range(4):
                    stg = padj.tile([128, 1024], i32, tag="adjstg", name="adjstg")
                    for b in range(2):
                        k = 2 * m + b
                        nc.sync.dma_start(
                            stg[:, b * 512:(b + 1) * 512],
                            adjT[k * 128:(k + 1) * 128, 0:512])
                    nc.vector.tensor_scalar(mask_pair[m][:], stg[:], 0, None,
                                            op0=OP.not_equal)
                for m in range(2):
                    stg = padj.tile([128, 1024], i32, tag="adjstg", name="adjstg")
                    for b in range(4):
                        k = 8 + 4 * m + b
                        nc.sync.dma_start(
                            stg[:, b * 256:(b + 1) * 256],
                            adjT[k * 128:(k + 1) * 128, 256:512])
                    nc.vector.tensor_scalar(mask_quad[m][:], stg[:], 0, None,
                                            op0=OP.not_equal)

                # row-emptiness is head-independent; filled from head 0's denom
                em_t = ppvs.tile([65, 512], f32r, tag="em_t", name="em_t")
                em_bc2 = ppvs.tile([128, 512], f32, tag="em_bc2", name="em_bc2")
                ones65 = ppvs.tile([65, 128], f32r, tag="ones65", name="ones65")
                nc.sync.dma_start(ones65[64:65, :], ones1)

                for g in range(HQ):
                    h = g // G
                    ti, slot = _qslot(g)
                    qg = qp_sb[ti][slot * 64:(slot + 1) * 64, :]
                    kh = kT_sb[h // 2][(h % 2) * 64:(h % 2) * 64 + 64, :]
                    pv = pspv.tile([65, 512], f32, tag="pv")
                    for m in range(4):      # s-block pairs, full n
                        simp = pssim.tile([128, 1024], f32, tag="sim")
                        for b in range(2):
                            k = 2 * m + b
                            nc.tensor.matmul(
                                simp[:, b * 512:(b + 1) * 512],
                                kh[:, k * 128:(k + 1) * 128], qg[:],
                                start=True, stop=True)
                        ex = pex.tile([128, 1024], f16, tag="ex")
                        nc.scalar.activation(ex[:], simp[:], AF.Exp)
                        nc.vector.tensor_tensor(ex[:], ex[:], mask_pair[m][:],
                                                op=OP.mult)
                        for b in range(2):
                            k = 2 * m + b
                            nc.tensor.matmul(
                                pv[:], v_ext[k][:, 65 * h:65 * h + 65],
                                ex[:, b * 512:(b + 1) * 512],
                                start=(k == 0), stop=False,
                                skip_group_check=True)
                    for m in range(2):      # s-block quads, n 256:512 only
                        simp = pssim.tile([128, 1024], f32, tag="sim")
                        for b in (0, 2, 1, 3):
                            k = 8 + 4 * m + b
                            nc.tensor.matmul(
                                simp[:, b * 256:(b + 1) * 256],
                                kh[:, k * 128:(k + 1) * 128], qg[:, 256:512],
                                start=True, stop=True)
                        ex = pex.tile([128, 1024], f16, tag="ex")
                        nc.scalar.activation(ex[:], simp[:], AF.Exp)
                        nc.vector.tensor_tensor(ex[:], ex[:], mask_quad[m][:],
                                                op=OP.mult)
                        for b in range(4):
                            k = 8 + 4 * m + b
                            nc.tensor.matmul(
                                pv[:, 256:512], v_ext[k][:, 65 * h:65 * h + 65],
                                ex[:, b * 256:(b + 1) * 256],
                                start=False, stop=(k == 15),
                                skip_group_check=True)

                    if g == 0:
                        nc.vector.tensor_scalar(em_t[64:65, :], pv[64:65, :], 0.0,
                                                None, op0=OP.is_equal)
                        bce = psbc.tile([128, 512], f32, tag="bce", name="bce")
                        nc.tensor.matmul(bce[:], ones65[64:65, :], em_t[64:65, :],
                                         start=True, stop=True)
                        nc.vector.tensor_copy(em_bc2[:], bce[:])
                    # den_safe = den + em  (both live at partition 64)
                    den = ptiny.tile([65, 512], f32r, tag="den")
                    nc.vector.tensor_tensor(den[64:65, :], pv[64:65, :],
                                            em_t[64:65, :], op=OP.add)
                    bc_r = psbc.tile([64, 512], f32, tag="bcr")
                    nc.tensor.matmul(bc_r[:], ones65[64:65, 0:64], den[64:65, :],
                                     start=True, stop=True)
                    rec = ppvs.tile([64, 512], f32, tag="rec")
                    nc.vector.reciprocal_approx_fast(rec[:], bc_r[:])
                    p0 = (g % 2) * 64
                    att = attT[g // 2][p0:p0 + 64, :]
                    nc.vector.tensor_tensor(att, pv[0:64, :], rec[:], op=OP.mult)
                    nc.vector.scalar_tensor_tensor(
                        att, em_bc2[p0:p0 + 64, :], mv2[h][p0:p0 + 64, :], att,
                        op0=OP.mult, op1=OP.add)

            # ---------------- phase 4: layernorm + out projection ----------------
            with (
                tc.tile_pool(name="lnt", bufs=2) as plnt,
                tc.tile_pool(name="ysb", bufs=2) as pysb,
                tc.tile_pool(name="psst", bufs=1, space="PSUM") as psst,
                tc.tile_pool(name="pslb", bufs=1, space="PSUM") as pslb,
                tc.tile_pool(name="psy", bufs=2, space="PSUM") as psy,
            ):
                st_sum = psst.tile([1, NL], f32, tag="ssum")
                st_sq = psst.tile([1, NL], f32, tag="ssq")
                for e in range(EC):
                    nc.tensor.matmul(st_sum[:], r(ones_m1[:]), r(attT[e][:]),
                                     start=(e == 0), stop=(e == EC - 1))
                    sq = plnt.tile([128, NL], f32r, tag="sq")
                    nc.scalar.activation(sq[:], attT[e][:], AF.Square)
                    nc.tensor.matmul(st_sq[:], r(ones_m1[:]), r(sq[:]),
                                     start=(e == 0), stop=(e == EC - 1))
                mu = plnt.tile([1, NL], f32r, tag="mu")
                nc.vector.tensor_scalar_mul(mu[:], st_sum[:], 1.0 / E)
                var = plnt.tile([1, NL], f32, tag="var")
                nc.vector.tensor_scalar_mul(var[:], st_sq[:], 1.0 / E)
                mu2 = plnt.tile([1, NL], f32, tag="mu2")
                nc.vector.tensor_tensor(mu2[:], mu[:], mu[:], op=OP.mult)
                nc.vector.tensor_tensor(var[:], var[:], mu2[:], op=OP.subtract)
                sd = plnt.tile([1, NL], f32r, tag="sd")
                nc.scalar.activation(sd[:], var[:], AF.Sqrt, bias=eps_c[:])
                nc.vector.reciprocal(sd[:], sd[:])
                mb = pslb.tile([128, NL], f32, tag="mb")
                nc.tensor.matmul(mb[:], r(ones_k1[:]), r(mu[:]), start=True, stop=True)
                ib = pslb.tile([128, NL], f32, tag="ib")
                nc.tensor.matmul(ib[:], r(ones_k1[:]), r(sd[:]), start=True, stop=True)
                for e in range(EC):
                    tmp = plnt.tile([128, NL], f32, tag="xn")
                    nc.vector.tensor_tensor(tmp[:], attT[e][:], mb[:], op=OP.subtract)
                    nc.vector.tensor_tensor(tmp[:], tmp[:], ib[:], op=OP.mult)
                    nc.vector.tensor_scalar(attT[e][:], tmp[:], lng_sb[e][:],
                                            lnb_sb[e][:], op0=OP.mult, op1=OP.add)
                for nt in range(4):
                    pyA = psy.tile([128, 512], f32, tag="pyA", name="pyA")
                    pyB = psy.tile([128, 512], f32, tag="pyB", name="pyB")
                    pys = (pyA, pyB)
                    for e in range(EC):
                        for oc in range(2):
                            nc.tensor.matmul(
                                pys[oc][:], r(attT[e][:, nt * 128:(nt + 1) * 128]),
                                r(wo_sb[e][:, oc * 512:(oc + 1) * 512]),
                                start=(e == 0), stop=False)
                    for oc in range(2):
                        nc.tensor.matmul(pys[oc][:], r(ones_k1[:]),
                                         r(bo_sb[0:1, oc * 512:(oc + 1) * 512]),
                                         start=False, stop=True)
                        ys = pysb.tile([128, 512], f32, tag="ys")
                        nc.vector.tensor_copy(ys[:], pys[oc][:])
                        nc.sync.dma_start(
                            y[nt * 128:(nt + 1) * 128, oc * 512:(oc + 1) * 512],
                            ys[:])
    nc.finalize()
    return nc


def shard_inputs(inputs):
    q = np.asarray(inputs["query"], np.float32)
    k = np.asarray(inputs["key"], np.float32)
    v = np.asarray(inputs["value"], np.float32)
    adj = np.asarray(inputs["adj"], np.int32)
    WqT8 = np.ascontiguousarray(np.asarray(inputs["Wq"], np.float32).T) / np.float32(8.0)
    WkT = np.ascontiguousarray(np.asarray(inputs["Wk"], np.float32).T)
    WvT = np.ascontiguousarray(np.asarray(inputs["Wv"], np.float32).T)
    WoT = np.ascontiguousarray(np.asarray(inputs["Wo"], np.float32).T)
    bq8 = (np.asarray(inputs["bq"], np.float32) / np.float32(8.0)).reshape(EC, 128)
    bk2 = np.asarray(inputs["bk"], np.float32).reshape(2, 128)
    bv2 = np.asarray(inputs["bv"], np.float32).reshape(2, 128)
    bo1 = np.asarray(inputs["bo"], np.float32).reshape(1, E)
    lng = np.asarray(inputs["ln_g"], np.float32).reshape(EC, 128)
    lnb = np.asarray(inputs["ln_b"], np.float32).reshape(EC, 128)

    shared = dict(WqT=WqT8, WkT=WkT, WvT=WvT, WoT=WoT, bq2d=bq8, bk2d=bk2,
                  bv2d=bv2, bo1=bo1, lng=lng, lnb=lnb,
                  ones1=np.ones((1, 128), np.float32))
    per_b = []
    s_idx = np.arange(N)
    for b in range(B):
        per_b.append((np.ascontiguousarray(k[b].T), np.ascontiguousarray(v[b].T)))
    in_maps = []
    for c in range(8):
        b, j = divmod(c, 4)
        rows = np.arange(j, N, 4)
        causal = s_idx[None, :] <= rows[:, None]          # [NL, N]
        adjc = np.where(causal, adj[b][rows], 0)
        m = dict(shared)
        m["xqT"] = np.ascontiguousarray(q[b][rows].T)
        m["xkT"], m["xvT"] = per_b[b]
        m["adjT"] = np.ascontiguousarray(adjc.T.astype(np.int32))
        in_maps.append(m)
    return in_maps


def unshard_outputs(results):
    out = np.empty((B, N, E), np.float32)
    for c in range(8):
        b, j = divmod(c, 4)
        out[b, j::4, :] = results[c]["y"]
    return out


def kernel(**inputs):
    from concourse.bass_utils import run_bass_kernel_spmd

    if "nc" not in _PROG_CACHE:
        _PROG_CACHE["nc"] = build_program()
    nc = _PROG_CACHE["nc"]
    in_maps = shard_inputs(inputs)
    res = run_bass_kernel_spmd(nc, in_maps, core_ids=list(range(8)))
    return unshard_outputs(res.results)


# revision 18
# speedup vs baseline: 1.3867x; 1.0047x over previous
"""GQA sparse-attention kernel for 8 Trainium2 NeuronCores.

Sharding: data-parallel over batch (2) x sequence-parallel over query rows
(4 row-groups per batch, rows j::4 interleaved so causal work is balanced and
the program is SPMD-identical across cores). No collectives: each core holds
512 query rows and computes all 16 heads for them, then layernorm + out-proj
for its rows locally.

Per-core device pipeline (all activations kept transposed, contraction dim on
partitions):
  qT = (WqT/8).T-chunks @ xqT   [1024,512]   (scale 1/8 folded into Wq host-side)
  kT = WkT-chunks @ xkT         [256,2048]
  vT = WvT-chunks @ xvT -> PE-transpose -> v_ext [s,260] (4 kv-heads x (64+ones))
  simT[s,n] = kT_h.T-slices @ qT_g   (fp32r, 4 s-blocks merged per PSUM tile)
  P = exp(simT) * mask01  (ACT exp -> bf16, DVE multiply; mask = adj&causal,
                           pre-masked on host, converted on device)
  pv[65,n] += v_ext_h.T @ P  (ones column gives the softmax denominator)
  att = num/denom; all-masked rows fall back to mean over all v rows (matches
  the reference's uniform-softmax-over-finfo.min behavior exactly)
  LN stats via ones-vector matmuls over the E-on-partitions layout, then
  y = xnorm.T-chunks @ WoT (+bo via a K=1 ones matmul)
"""

import os
import sys

import numpy as np

for _p in ("/opt/trn_rl_repo", "/root/.axon_site/_ro/trn_rl_repo"):
    if os.path.isdir(_p) and _p not in sys.path:
        sys.path.insert(0, _p)

B, N, E = 2, 2048, 1024
HQ, HK, D = 16, 4, 64
G = HQ // HK          # 4 query heads per kv head
KVE = HK * D          # 256
NL = N // 4           # 512 local query rows per core
SB = N // 128         # 16 s-blocks
EC = E // 128         # 8 embedding chunks
LN_EPS = 1e-5
NEG = 0.0  # host pre-masks adj; no additive mask needed

_PROG_CACHE = {}


def build_program():
    import concourse.bass as bass
    import concourse.mybir as mybir
    import concourse.tile as tile
    from concourse import bacc

    dt = mybir.dt
    f32, f32r, bf16, i32 = dt.float32, dt.float32r, dt.bfloat16, dt.int32
    f16 = dt.float16
    AF = mybir.ActivationFunctionType
    OP = mybir.AluOpType
    AX = mybir.AxisListType

    nc = bacc.Bacc("TRN2", target_bir_lowering=False, debug=False)

    def din(name, shape, dtp=f32):
        return nc.dram_tensor(name, shape, dtp, kind="ExternalInput").ap()

    xqT = din("xqT", [E, NL], f16)
    xkT = din("xkT", [E, N], f16)
    xvT = din("xvT", [E, N], f16)
    adjT = din("adjT", [N, NL], i32)
    WqT = din("WqT", [E, E], f16)          # pre-scaled by 1/8 on host
    WkT = din("WkT", [E, KVE], f16)
    WvT = din("WvT", [E, KVE], f16)
    WoT = din("WoT", [E, E], f16)
    bq2d = din("bq2d", [EC, 128])     # bq/8
    bk2d = din("bk2d", [2, 128])
    bv2d = din("bv2d", [2, 128])
    bo1 = din("bo1", [1, E], f16)
    lng = din("lng", [EC, 128])
    lnb = din("lnb", [EC, 128])
    ones1 = din("ones1", [1, 128], f32r)
    ones1h = din("ones1h", [1, 128], f16)
    y = nc.dram_tensor("y", [NL, E], f32, kind="ExternalOutput").ap()

    r = lambda ap: ap  # tiles feeding fp32r matmuls are float32r-typed

    with tile.TileContext(nc) as tc, nc.allow_low_precision(
            "float32r operands for PE fast-path matmuls are intentional"):
        with (
            tc.tile_pool(name="const", bufs=1) as pc,
            tc.tile_pool(name="persist", bufs=1) as pp,
            tc.tile_pool(name="psc", bufs=1, space="PSUM") as psc,
        ):
            ident = pc.tile([128, 128], f16, tag="ident")
            from concourse.masks import make_identity
            make_identity(nc, ident[:])
            ones_k1 = pc.tile([1, 128], f32r, tag="ones_k1")
            nc.sync.dma_start(ones_k1[:], ones1)
            ones_m1 = pc.tile([128, 1], f16, tag="ones_m1")
            nc.sync.dma_start(ones_m1[:], ones1h)
            ones_k1h = pc.tile([1, 128], f16, tag="ones_k1h")
            nc.sync.dma_start(ones_k1h[:], ones1h)
            eps_c = pc.tile([1, 1], f32, tag="eps_c")
            nc.gpsimd.memset(eps_c[:], LN_EPS)

            # persistent tiles
            wo_sb = [pp.tile([128, E], f16, tag=f"wo{e}", name=f"wo{e}") for e in range(EC)]
            for e in range(EC):
                nc.sync.dma_start(wo_sb[e][:], WoT[e * 128:(e + 1) * 128, :])
            bo_sb = pp.tile([1, E], f16, tag="bo", name="bo")
            nc.sync.dma_start(bo_sb[:], bo1)
            lng_sb = [pp.tile([128, 1], f32, tag=f"lng{e}", name=f"lng{e}") for e in range(EC)]
            lnb_sb = [pp.tile([128, 1], f32, tag=f"lnb{e}", name=f"lnb{e}") for e in range(EC)]
            for e in range(EC):
                nc.sync.dma_start(lng_sb[e][:], lng[e:e + 1, :])
                nc.sync.dma_start(lnb_sb[e][:], lnb[e:e + 1, :])

            kT_sb = [pp.tile([128, N], f16, tag=f"kt{m}", name=f"kt{m}") for m in range(2)]
            v_ext = [pp.tile([128, 4 * 128], f16, tag=f"vx{k}", name=f"vx{k}") for k in range(SB)]
            meanv = [pp.tile([128, 1], f32, tag=f"mv{m}", name=f"mv{m}") for m in range(2)]
            mv2 = [pp.tile([128, 1], f32, tag=f"mv2{h}", name=f"mv2{h}") for h in range(HK)]
            # q head-pair tiles: half 0 holds a head with even kv-head, half 1 odd,
            # so sim matmul operand partition bases match the kv-head's base in kT_sb.
            qp_sb = [pp.tile([128, NL], f16, tag=f"qp{m}", name=f"qp{m}") for m in range(EC)]
            _EVEN = [0, 1, 2, 3, 8, 9, 10, 11]    # heads with (g//4) % 2 == 0
            _ODD = [4, 5, 6, 7, 12, 13, 14, 15]
            def _qslot(g):
                if (g // G) % 2 == 0:
                    return _EVEN.index(g), 0
                return _ODD.index(g), 1
            attT = [pp.tile([128, NL], f16, tag=f"at{e}", name=f"at{e}") for e in range(EC)]
            # merged mask tiles: 4 pair tiles (s-blocks 2m,2m+1 over n 0:512) and
            # 2 quad tiles (s-blocks 8+4m..11+4m over n 256:512)
            mask_pair = [pp.tile([128, 1024], f16, tag=f"mkp{m}", name=f"mkp{m}") for m in range(4)]
            mask_quad = [pp.tile([128, 1024], f16, tag=f"mkq{m}", name=f"mkq{m}") for m in range(2)]

            # ---------------- phase 1: q projection ----------------
            with (
                tc.tile_pool(name="wq", bufs=1) as pwq,
                tc.tile_pool(name="xq", bufs=1) as pxq,
                tc.tile_pool(name="bq", bufs=1) as pbq,
                tc.tile_pool(name="psq", bufs=2, space="PSUM") as psq,
            ):
                wq_sb = [pwq.tile([128, E], f16, tag=f"wq{e}", name=f"wq{e}") for e in range(EC)]
                xq_sb = [pxq.tile([128, NL], f16, tag=f"xq{e}", name=f"xq{e}") for e in range(EC)]
                bq_sb = [pbq.tile([128, 1], f32, tag=f"bq{m}", name=f"bq{m}") for m in range(EC)]
                for e in range(EC):
                    nc.sync.dma_start(wq_sb[e][:], WqT[e * 128:(e + 1) * 128, :])
                    nc.sync.dma_start(xq_sb[e][:], xqT[e * 128:(e + 1) * 128, :])
                    nc.sync.dma_start(bq_sb[e][:], bq2d[e:e + 1, :])
                for mt2 in range(EC // 2):
                    psA = psq.tile([128, NL], f32, tag="psqA", name="psqA")
                    psB = psq.tile([128, NL], f32, tag="psqB", name="psqB")
                    for e in range(EC):
                        for mt, ps in ((2 * mt2, psA), (2 * mt2 + 1, psB)):
                            nc.tensor.matmul(
                                ps[:], r(wq_sb[e][:, mt * 128:(mt + 1) * 128]),
                                r(xq_sb[e][:]), start=(e == 0), stop=(e == EC - 1))
                    for mt, ps in ((2 * mt2, psA), (2 * mt2 + 1, psB)):
                     for t in range(2):
                        g = 2 * mt + t
                        ti, slot = _qslot(g)
                        nc.scalar.activation(
                            qp_sb[ti][slot * 64:(slot + 1) * 64, :],
                            ps[t * 64:(t + 1) * 64, :], AF.Identity,
                            bias=bq_sb[mt][t * 64:(t + 1) * 64, :], scale=1.0)

            # ---------------- phase 2: k/v projections ----------------
            with (
                tc.tile_pool(name="wkv", bufs=1) as pwkv,
                tc.tile_pool(name="xkv", bufs=3) as pxkv,
                tc.tile_pool(name="vt", bufs=2) as pvt,
                tc.tile_pool(name="vs", bufs=1) as pvs,
                tc.tile_pool(name="pskv", bufs=2, space="PSUM") as pskv,
                tc.tile_pool(name="pst", bufs=2, space="PSUM") as pst,
            ):
                wk_sb = [pwkv.tile([128, KVE], f16, tag=f"wk{e}", name=f"wk{e}") for e in range(EC)]
                wv_sb = [pwkv.tile([128, KVE], f16, tag=f"wv{e}", name=f"wv{e}") for e in range(EC)]
                bk_sb = [pwkv.tile([128, 1], f32, tag=f"bk{m}", name=f"bk{m}") for m in range(2)]
                bv_sb = [pwkv.tile([128, 1], f32, tag=f"bv{m}", name=f"bv{m}") for m in range(2)]
                for e in range(EC):
                    nc.sync.dma_start(wk_sb[e][:], WkT[e * 128:(e + 1) * 128, :])
                    nc.sync.dma_start(wv_sb[e][:], WvT[e * 128:(e + 1) * 128, :])
                for m in range(2):
                    nc.sync.dma_start(bk_sb[m][:], bk2d[m:m + 1, :])
                    nc.sync.dma_start(bv_sb[m][:], bv2d[m:m + 1, :])
                vsum = [pvs.tile([128, 4], f32, tag=f"vsum{m}", name=f"vsum{m}") for m in range(2)]
                for st in range(4):  # s-tiles of 512
                    sl = slice(st * 512, (st + 1) * 512)
                    xk_sb = [pxkv.tile([128, 512], f16, tag=f"xk{e % 2}", name=f"xk{e}") for e in range(EC)]
                    xv_sb = [pxkv.tile([128, 512], f16, tag=f"xv{e % 2}", name=f"xv{e}") for e in range(EC)]
                    for e in range(EC):
                        nc.sync.dma_start(xk_sb[e][:], xkT[e * 128:(e + 1) * 128, sl])
                        nc.sync.dma_start(xv_sb[e][:], xvT[e * 128:(e + 1) * 128, sl])
                    for mt in range(2):
                        psk = pskv.tile([128, 512], f32, tag="psk")
                        psv = pskv.tile([128, 512], f32, tag="psv")
                        for e in range(EC):
                            nc.tensor.matmul(
                                psk[:], r(wk_sb[e][:, mt * 128:(mt + 1) * 128]),
                                r(xk_sb[e][:]), start=(e == 0), stop=(e == EC - 1))
                            nc.tensor.matmul(
                                psv[:], r(wv_sb[e][:, mt * 128:(mt + 1) * 128]),
                                r(xv_sb[e][:]), start=(e == 0), stop=(e == EC - 1))
                        nc.scalar.activation(kT_sb[mt][:, sl], psk[:], AF.Identity,
                                             bias=bk_sb[mt][:], scale=1.0)
                        vt = pvt.tile([128, 512], f16, tag="vt")
                        nc.scalar.activation(vt[:], psv[:], AF.Identity,
                                             bias=bv_sb[mt][:], scale=1.0)
                        nc.vector.reduce_sum(vsum[mt][:, st:st + 1], vt[:], axis=AX.X)
                        for ss in range(4):
                            k = st * 4 + ss
                            pt = pst.tile([128, 128], f16, tag="pt")
                            nc.tensor.transpose(pt[:], vt[:, ss * 128:(ss + 1) * 128],
                                                ident[:])
                            src = pt[:].rearrange("p (h x) -> p h x", h=2)
                            dst = v_ext[k][:].rearrange("p (h x) -> p h x", h=4)
                            nc.vector.tensor_copy(dst[:, 2 * mt:2 * mt + 2, 0:64], src)
                for k in range(SB):
                    pad = v_ext[k][:].rearrange("p (h x) -> p h x", h=4)[:, :, 64:128]
                    nc.gpsimd.memset(pad, 0.0)
                    one_col = v_ext[k][:].rearrange("p (h x) -> p h x", h=4)[:, :, 64:65]
                    nc.gpsimd.memset(one_col, 1.0)
                for m in range(2):
                    nc.vector.tensor_reduce(meanv[m][:], vsum[m][:], axis=AX.X,
                                            op=OP.add)
                    nc.vector.tensor_scalar_mul(meanv[m][:], meanv[m][:], 1.0 / N)
                for h in range(HK):
                    src = meanv[h // 2][(h % 2) * 64:(h % 2) * 64 + 64, :]
                    nc.vector.tensor_copy(mv2[h][0:64, :], src)
                    nc.vector.tensor_copy(mv2[h][64:128, :], src)

            # ---------------- phase 3: attention ----------------
            with (
                tc.tile_pool(name="adjs", bufs=2) as padj,
                tc.tile_pool(name="exps", bufs=3) as pex,
                tc.tile_pool(name="pvs_sb", bufs=2) as ppvs,
                tc.tile_pool(name="tiny", bufs=2) as ptiny,
                tc.tile_pool(name="pssim", bufs=2, space="PSUM") as pssim,
                tc.tile_pool(name="pspv", bufs=2, space="PSUM") as pspv,
                tc.tile_pool(name="psbc", bufs=1, space="PSUM") as psbc,
            ):
                # build merged masks
                for m in range(4):
                    stg = padj.tile([128, 1024], i32, tag="adjstg", name="adjstg")
                    for b in range(2):
                        k = 2 * m + b
                        nc.sync.dma_start(
                            stg[:, b * 512:(b + 1) * 512],
                            adjT[k * 128:(k + 1) * 128, 0:512])
                    nc.vector.tensor_scalar(mask_pair[m][:], stg[:], 0, None,
                                            op0=OP.not_equal)
                for m in range(2):
                    stg = padj.tile([128, 1024], i32, tag="adjstg", name="adjstg")
                    for b in range(4):
                        k = 8 + 4 * m + b
                        nc.sync.dma_start(
                            stg[:, b * 256:(b + 1) * 256],
                            adjT[k * 128:(k + 1) * 128, 256:512])
                    nc.vector.tensor_scalar(mask_quad[m][:], stg[:], 0, None,
                                            op0=OP.not_equal)

                # row-emptiness is head-independent; filled from head 0's denom
                em_t = ppvs.tile([65, 512], f32r, tag="em_t", name="em_t")
                em_bc2 = ppvs.tile([128, 512], f32, tag="em_bc2", name="em_bc2")
                ones65 = ppvs.tile([65, 128], f32r, tag="ones65", name="ones65")
                nc.sync.dma_start(ones65[64:65, :], ones1)

                for g in range(HQ):
                    h = g // G
                    ti, slot = _qslot(g)
                    qg = qp_sb[ti][slot * 64:(slot + 1) * 64, :]
                    kh = kT_sb[h // 2][(h % 2) * 64:(h % 2) * 64 + 64, :]
                    pv = pspv.tile([128, 512], f32, tag="pv")
                    for m in range(4):      # s-block pairs, full n
                        simp = pssim.tile([128, 1024], f32, tag="sim")
                        for b in range(2):
                            k = 2 * m + b
                            nc.tensor.matmul(
                                simp[:, b * 512:(b + 1) * 512],
                                kh[:, k * 128:(k + 1) * 128], qg[:],
                                start=True, stop=True)
                        ex = pex.tile([128, 1024], f16, tag="ex")
                        nc.scalar.activation(ex[:], simp[:], AF.Exp)
                        nc.vector.tensor_tensor(ex[:], ex[:], mask_pair[m][:],
                                                op=OP.mult)
                        for b in range(2):
                            k = 2 * m + b
                            nc.tensor.matmul(
                                pv[:], v_ext[k][:, 128 * h:128 * h + 128],
                                ex[:, b * 512:(b + 1) * 512],
                                start=(k == 0), stop=False,
                                skip_group_check=True)
                    for m in range(2):      # s-block quads, n 256:512 only
                        simp = pssim.tile([128, 1024], f32, tag="sim")
                        for b in (0, 2, 1, 3):
                            k = 8 + 4 * m + b
                            nc.tensor.matmul(
                                simp[:, b * 256:(b + 1) * 256],
                                kh[:, k * 128:(k + 1) * 128], qg[:, 256:512],
                                start=True, stop=True)
                        ex = pex.tile([128, 1024], f16, tag="ex")
                        nc.scalar.activation(ex[:], simp[:], AF.Exp)
                        nc.vector.tensor_tensor(ex[:], ex[:], mask_quad[m][:],
                                                op=OP.mult)
                        for b in range(4):
                            k = 8 + 4 * m + b
                            nc.tensor.matmul(
                                pv[:, 256:512], v_ext[k][:, 128 * h:128 * h + 128],
                                ex[:, b * 256:(b + 1) * 256],
                                start=False, stop=(k == 15),
                                skip_group_check=True)

                    if g == 0:
                        nc.vector.tensor_scalar(em_t[64:65, :], pv[64:65, :], 0.0,
                                                None, op0=OP.is_equal)
                        bce = psbc.tile([128, 512], f32, tag="bce", name="bce")
                        nc.tensor.matmul(bce[:], ones65[64:65, :], em_t[64:65, :],
                                         start=True, stop=True)
                        nc.vector.tensor_copy(em_bc2[:], bce[:])
                    # den_safe = den + em  (both live at partition 64)
                    den = ptiny.tile([65, 512], f32r, tag="den")
                    nc.vector.tensor_tensor(den[64:65, :], pv[64:65, :],
                                            em_t[64:65, :], op=OP.add)
                    bc_r = psbc.tile([64, 512], f32, tag="bcr")
                    nc.tensor.matmul(bc_r[:], ones65[64:65, 0:64], den[64:65, :],
                                     start=True, stop=True)
                    rec = ppvs.tile([64, 512], f32, tag="rec")
                    nc.vector.reciprocal_approx_fast(rec[:], bc_r[:])
                    p0 = (g % 2) * 64
                    att = attT[g // 2][p0:p0 + 64, :]
                    nc.vector.tensor_tensor(att, pv[0:64, :], rec[:], op=OP.mult)
                    nc.vector.scalar_tensor_tensor(
                        att, em_bc2[p0:p0 + 64, :], mv2[h][p0:p0 + 64, :], att,
                        op0=OP.mult, op1=OP.add)

            # ---------------- phase 4: layernorm + out projection ----------------
            with (
                tc.tile_pool(name="lnt", bufs=2) as plnt,
                tc.tile_pool(name="ysb", bufs=2) as pysb,
                tc.tile_pool(name="psst", bufs=1, space="PSUM") as psst,
                tc.tile_pool(name="pslb", bufs=1, space="PSUM") as pslb,
                tc.tile_pool(name="psy", bufs=2, space="PSUM") as psy,
            ):
                st_sum = psst.tile([1, NL], f32, tag="ssum")
                st_sq = psst.tile([1, NL], f32, tag="ssq")
                for e in range(EC):
                    nc.tensor.matmul(st_sum[:], r(ones_m1[:]), r(attT[e][:]),
                                     start=(e == 0), stop=(e == EC - 1))
                    sq = plnt.tile([128, NL], f16, tag="sq")
                    nc.scalar.activation(sq[:], attT[e][:], AF.Square)
                    nc.tensor.matmul(st_sq[:], r(ones_m1[:]), r(sq[:]),
                                     start=(e == 0), stop=(e == EC - 1))
                mu = plnt.tile([1, NL], f32r, tag="mu")
                nc.vector.tensor_scalar_mul(mu[:], st_sum[:], 1.0 / E)
                var = plnt.tile([1, NL], f32, tag="var")
                nc.vector.tensor_scalar_mul(var[:], st_sq[:], 1.0 / E)
                mu2 = plnt.tile([1, NL], f32, tag="mu2")
                nc.vector.tensor_tensor(mu2[:], mu[:], mu[:], op=OP.mult)
                nc.vector.tensor_tensor(var[:], var[:], mu2[:], op=OP.subtract)
                sd = plnt.tile([1, NL], f32r, tag="sd")
                nc.scalar.activation(sd[:], var[:], AF.Sqrt, bias=eps_c[:])
                nc.vector.reciprocal(sd[:], sd[:])
                mb = pslb.tile([128, NL], f32, tag="mb")
                nc.tensor.matmul(mb[:], r(ones_k1[:]), r(mu[:]), start=True, stop=True)
                ib = pslb.tile([128, NL], f32, tag="ib")
                nc.tensor.matmul(ib[:], r(ones_k1[:]), r(sd[:]), start=True, stop=True)
                for e in range(EC):
                    tmp = plnt.tile([128, NL], f32, tag="xn")
                    nc.vector.tensor_tensor(tmp[:], attT[e][:], mb[:], op=OP.subtract)
                    nc.vector.tensor_tensor(tmp[:], tmp[:], ib[:], op=OP.mult)
                    nc.vector.tensor_scalar(attT[e][:], tmp[:], lng_sb[e][:],
                                            lnb_sb[e][:], op0=OP.mult, op1=OP.add)
                for nt in range(4):
                    pyA = psy.tile([128, 512], f32, tag="pyA", name="pyA")
                    pyB = psy.tile([128, 512], f32, tag="pyB", name="pyB")
                    pys = (pyA, pyB)
                    for e in range(EC):
                        for oc in range(2):
                            nc.tensor.matmul(
                                pys[oc][:], r(attT[e][:, nt * 128:(nt + 1) * 128]),
                                r(wo_sb[e][:, oc * 512:(oc + 1) * 512]),
                                start=(e == 0), stop=False)
                    for oc in range(2):
                        nc.tensor.matmul(pys[oc][:], ones_k1h[:],
                                         bo_sb[0:1, oc * 512:(oc + 1) * 512],
                                         start=False, stop=True)
                        ys = pysb.tile([128, 512], f32, tag="ys")
                        nc.vector.tensor_copy(ys[:], pys[oc][:])
                        nc.sync.dma_start(
                            y[nt * 128:(nt + 1) * 128, oc * 512:(oc + 1) * 512],
                            ys[:])
    nc.finalize()
    return nc


def shard_inputs(inputs):
    q = np.asarray(inputs["query"], np.float32)
    k = np.asarray(inputs["key"], np.float32)
    v = np.asarray(inputs["value"], np.float32)
    adj = np.asarray(inputs["adj"], np.int32)
    WqT8 = (np.ascontiguousarray(np.asarray(inputs["Wq"], np.float32).T)
            / np.float32(8.0)).astype(np.float16)
    WkT = np.ascontiguousarray(np.asarray(inputs["Wk"], np.float32).T).astype(np.float16)
    WvT = np.ascontiguousarray(np.asarray(inputs["Wv"], np.float32).T).astype(np.float16)
    WoT = np.ascontiguousarray(np.asarray(inputs["Wo"], np.float32).T).astype(np.float16)
    bq8 = (np.asarray(inputs["bq"], np.float32) / np.float32(8.0)).reshape(EC, 128)
    bk2 = np.asarray(inputs["bk"], np.float32).reshape(2, 128)
    bv2 = np.asarray(inputs["bv"], np.float32).reshape(2, 128)
    bo1 = np.asarray(inputs["bo"], np.float32).reshape(1, E).astype(np.float16)
    lng = np.asarray(inputs["ln_g"], np.float32).reshape(EC, 128)
    lnb = np.asarray(inputs["ln_b"], np.float32).reshape(EC, 128)

    shared = dict(WqT=WqT8, WkT=WkT, WvT=WvT, WoT=WoT, bq2d=bq8, bk2d=bk2,
                  bv2d=bv2, bo1=bo1, lng=lng, lnb=lnb,
                  ones1=np.ones((1, 128), np.float32),
                  ones1h=np.ones((1, 128), np.float16))
    per_b = []
    s_idx = np.arange(N)
    for b in range(B):
        per_b.append((np.ascontiguousarray(k[b].T).astype(np.float16),
                      np.ascontiguousarray(v[b].T).astype(np.float16)))
    in_maps = []
    for c in range(8):
        b, j = divmod(c, 4)
        rows = np.arange(j, N, 4)
        causal = s_idx[None, :] <= rows[:, None]          # [NL, N]
        adjc = np.where(causal, adj[b][rows], 0)
        m = dict(shared)
        m["xqT"] = np.ascontiguousarray(q[b][rows].T).astype(np.float16)
        m["xkT"], m["xvT"] = per_b[b]
        m["adjT"] = np.ascontiguousarray(adjc.T.astype(np.int32))
        in_maps.append(m)
    return in_maps


def unshard_outputs(results):
    out = np.empty((B, N, E), np.float32)
    for c in range(8):
        b, j = divmod(c, 4)
        out[b, j::4, :] = results[c]["y"]
    return out


def kernel(**inputs):
    from concourse.bass_utils import run_bass_kernel_spmd

    if "nc" not in _PROG_CACHE:
        _PROG_CACHE["nc"] = build_program()
    nc = _PROG_CACHE["nc"]
    in_maps = shard_inputs(inputs)
    res = run_bass_kernel_spmd(nc, in_maps, core_ids=list(range(8)))
    return unshard_outputs(res.results)
